# revision 10
# baseline (speedup 1.0000x reference)
"""Trainium2 Bass kernel for nn_DglGraphAttentionNetwork (GAT layer over a
random graph, B=16, L=1024, DIN=512, H=4 heads, DH=128).

Strategy (8 NeuronCores, SPMD):
  Launch A (data-parallel over nodes): each core projects its 2048 nodes
    (hidden = text@W, h = hidden@fc_w, el/er attention dots) and emits the
    h rows as a bf16 gather table [2048, 512] plus el/er [8, 2048].
  Host: concatenates the 8 table slices into the full [16384, 512] table
    and computes the per-edge softmax weights alpha from el/er (cheap
    scalar pipeline over the edge list; edges are pre-sorted by dst).
  Launch B (dst-sharded): edges are sorted by destination and grouped into
    128-destination blocks (host balances blocks by relabeling nodes). Each
    core handles 16 blocks: h[src] rows are fetched with indirect dma_gather
    (128 edges across partitions), scaled by the uploaded alpha, and the
    per-destination segment-sum is a masked matmul accumulating in PSUM.
"""

import os
import sys

sys.path.insert(0, "/opt/trn_rl_repo")

from contextlib import ExitStack

import numpy as np
import ml_dtypes

import jax
from jax.sharding import Mesh, PartitionSpec
from jax.experimental.shard_map import shard_map

try:
    jax.config.update("jax_compilation_cache_dir", "/tmp/gat_jax_cache")
    jax.config.update("jax_persistent_cache_min_compile_time_secs", 1.0)
    jax.config.update("jax_persistent_cache_min_entry_size_bytes", -1)
except Exception:
    pass

import concourse.bass as bass
import concourse.bacc as bacc
import concourse.mybir as mybir
import concourse.tile as tile
from concourse import library_config
from concourse import bass2jax
from concourse.bass2jax import _bass_exec_p, install_neuronx_cc_hook, partition_id_tensor

F32 = mybir.dt.float32
F32R = mybir.dt.float32r
BF16 = mybir.dt.bfloat16
I16 = mybir.dt.int16

B, L, DIN = 16, 1024, 512
H, DH = 4, 128
N = B * L           # 16384 nodes
NC = 8              # cores
NPC = N // NC       # 2048 nodes per core
NBLK = 128          # destination blocks of 128 nodes
BPC = NBLK // NC    # 16 blocks per core
NEG = 0.2           # leaky_relu slope
ELEM = 512          # gather-table row: just the h features (bf16 -> 1024B)

GDT = BF16
GNP = ml_dtypes.bfloat16
# phase-A matmul dtype: float32r streams at bf16 rate with fp32 operands,
# but requires f32r-rounding producers (see build_phase_a).
MM_F32R = os.environ.get("GAT_MMDT", "f32r") == "f32r"
MMDT = F32R if MM_F32R else F32


# ----------------------------------------------------------------------------
# Launch A: projection. Per core: textT [512, 2048] -> table [2048, 512], elr.
# ----------------------------------------------------------------------------

def build_phase_a():
    nc = bacc.Bacc("TRN2", target_bir_lowering=False, debug=False,
                   enable_asserts=False, num_devices=NC)
    textT = nc.dram_tensor("textT", [DIN, NPC], F32, kind="ExternalInput").ap()
    weight = nc.dram_tensor("weight", [DIN, DIN], F32, kind="ExternalInput").ap()
    fc_w = nc.dram_tensor("fc_w", [DIN, DIN], F32, kind="ExternalInput").ap()
    attn = nc.dram_tensor("attn", [DIN, 2 * H], F32, kind="ExternalInput").ap()
    ident = nc.dram_tensor("ident", [128, 128], F32, kind="ExternalInput").ap()
    bias_in = nc.dram_tensor("biasv", [DIN, 1], F32, kind="ExternalInput").ap()
    table = nc.dram_tensor("table", [NPC, ELEM], GDT, kind="ExternalOutput").ap()
    elr = nc.dram_tensor("elr", [2 * H, NPC], F32, kind="ExternalOutput").ap()

    KT = DIN // 128  # 4 contraction tiles

    with tile.TileContext(nc) as tc, ExitStack() as ctx:
        wpool = ctx.enter_context(tc.tile_pool(name="w", bufs=1))
        hpool = ctx.enter_context(tc.tile_pool(name="h", bufs=1))
        tpool = ctx.enter_context(tc.tile_pool(name="t", bufs=3))
        pmm = ctx.enter_context(tc.tile_pool(name="pmm", bufs=2, space="PSUM"))
        pelr = ctx.enter_context(tc.tile_pool(name="pelr", bufs=2, space="PSUM"))
        ptr = ctx.enter_context(tc.tile_pool(name="ptr", bufs=2, space="PSUM"))

        # Load via DMA then launder through one DVE copy each: fp32 matmuls
        # carry a single sync-wait slot in codegen, so every matmul operand
        # must be produced by the same engine (DVE) rather than by one of
        # the 8 round-robin DMA semaphore lanes.
        w_ld = [wpool.tile([128, DIN], F32, tag=f"wl{i}", name=f"wl{i}") for i in range(KT)]
        fc_ld = [wpool.tile([128, DIN], F32, tag=f"fl{i}", name=f"fl{i}") for i in range(KT)]
        attn_ld = [wpool.tile([128, 2 * H], F32, tag=f"al{i}", name=f"al{i}") for i in range(KT)]
        tT_ld = [wpool.tile([128, NPC], F32, tag=f"tl{i}", name=f"tl{i}") for i in range(KT)]
        id_ld = wpool.tile([128, 128], F32, tag="idl", name="idl")
        for i in range(KT):
            nc.sync.dma_start(w_ld[i][:], weight[i * 128:(i + 1) * 128, :])
            nc.sync.dma_start(fc_ld[i][:], fc_w[i * 128:(i + 1) * 128, :])
            nc.sync.dma_start(attn_ld[i][:], attn[i * 128:(i + 1) * 128, :])
            nc.sync.dma_start(tT_ld[i][:], textT[i * 128:(i + 1) * 128, :])
        nc.sync.dma_start(id_ld[:], ident[:])
        # bias[f] laid out feature-on-partition: col t holds bias[t*128+p]
        bias_sb = wpool.tile([128, KT], F32, tag="bv", name="bv")
        nc.sync.dma_start(bias_sb[:], bias_in.rearrange("(t p) o -> p (t o)", p=128))
        w_sb = [wpool.tile([128, DIN], MMDT, tag=f"w{i}", name=f"w{i}") for i in range(KT)]
        fc_sb = [wpool.tile([128, DIN], MMDT, tag=f"fc{i}", name=f"fc{i}") for i in range(KT)]
        attn_sb = [wpool.tile([128, 2 * H], MMDT, tag=f"at{i}", name=f"at{i}") for i in range(KT)]
        tT_sb = [wpool.tile([128, NPC], MMDT, tag=f"tt{i}", name=f"tt{i}") for i in range(KT)]
        id_sb = wpool.tile([128, 128], MMDT, tag="id", name="id")
        for i in range(KT):
            nc.vector.tensor_copy(w_sb[i][:], w_ld[i][:])
            nc.vector.tensor_copy(fc_sb[i][:], fc_ld[i][:])
            nc.vector.tensor_copy(attn_sb[i][:], attn_ld[i][:])
            nc.vector.tensor_copy(tT_sb[i][:], tT_ld[i][:])
        nc.vector.tensor_copy(id_sb[:], id_ld[:])

        # hiddenT[e, n] = sum_d W[d, e] * textT[d, n]
        h1_sb = [hpool.tile([128, NPC], MMDT, tag=f"h1{i}", name=f"h1{i}") for i in range(KT)]
        for et in range(KT):
            for nch in range(NPC // 512):
                p = pmm.tile([128, 512], F32, tag="pmm", name="pmm")
                for dt in range(KT):
                    nc.tensor.matmul(
                        p[:],
                        w_sb[dt][:, et * 128:(et + 1) * 128],
                        tT_sb[dt][:, nch * 512:(nch + 1) * 512],
                        start=(dt == 0), stop=(dt == KT - 1))
                nc.vector.tensor_copy(h1_sb[et][:, nch * 512:(nch + 1) * 512], p[:])

        # hT[f, n] = sum_e fc_w[e, f] * hiddenT[e, n]
        h2_sb = [hpool.tile([128, NPC], MMDT, tag=f"h2{i}", name=f"h2{i}") for i in range(KT)]
        for ft in range(KT):
            for nch in range(NPC // 512):
                p = pmm.tile([128, 512], F32, tag="pmm", name="pmm")
                for et in range(KT):
                    nc.tensor.matmul(
                        p[:],
                        fc_sb[et][:, ft * 128:(ft + 1) * 128],
                        h1_sb[et][:, nch * 512:(nch + 1) * 512],
                        start=(et == 0), stop=(et == KT - 1))
                # + bias here: out = sum_e alpha_e (h[src]+bias) = agg + bias
                # since softmax weights sum to 1; the el/er shift it also
                # induces is constant per head, which softmax cancels.
                nc.vector.tensor_scalar(
                    h2_sb[ft][:, nch * 512:(nch + 1) * 512], p[:],
                    bias_sb[:, ft:ft + 1], None, op0=mybir.AluOpType.add)

        # elrT[c, n] (c = 4 el heads then 4 er heads)
        elr_sb = hpool.tile([2 * H, NPC], F32, tag="elr", name="elr")
        for nch in range(NPC // 512):
            p = pelr.tile([2 * H, 512], F32, tag="pelr", name="pelr")
            for ft in range(KT):
                nc.tensor.matmul(
                    p[:],
                    attn_sb[ft][:],
                    h2_sb[ft][:, nch * 512:(nch + 1) * 512],
                    start=(ft == 0), stop=(ft == KT - 1))
            nc.vector.tensor_copy(elr_sb[:, nch * 512:(nch + 1) * 512], p[:])
        nc.sync.dma_start(elr[:], elr_sb[:])

        # per 128-node tile: transpose hT into row-major table rows
        for nt in range(NPC // 128):
            tab = tpool.tile([128, ELEM], GDT, tag="tab", name="tab")
            for ft in range(KT):
                pt = ptr.tile([128, 128], MMDT, tag="ptr", name="ptr")
                nc.tensor.transpose(
                    pt[:], h2_sb[ft][:, nt * 128:(nt + 1) * 128], id_sb[:])
                nc.scalar.activation(
                    tab[:, ft * DH:(ft + 1) * DH], pt[:],
                    mybir.ActivationFunctionType.Copy)
            nc.sync.dma_start(table[nt * 128:(nt + 1) * 128, :], tab[:])
    nc.compile()
    return nc


# ----------------------------------------------------------------------------
# Launch B: weighted aggregation (alpha precomputed on host), dst-sharded.
# ----------------------------------------------------------------------------

def build_phase_b(s_max: int):
    p_b = s_max * 128          # padded edges per block
    npad = BPC * p_b           # padded edges per core

    nc = bacc.Bacc("TRN2", target_bir_lowering=False, debug=False,
                   enable_asserts=False, num_devices=NC)
    table = nc.dram_tensor("table", [N, ELEM], GDT, kind="ExternalInput").ap()
    idx_in = nc.dram_tensor("idx16", [128, npad // 16], I16, kind="ExternalInput").ap()
    dcol_c = nc.dram_tensor("dcolc", [128, BPC * s_max], BF16, kind="ExternalInput").ap()
    alf_in = nc.dram_tensor("alf", [128, BPC * s_max * H], BF16, kind="ExternalInput").ap()
    iota_r = nc.dram_tensor("iotar", [128, 128], BF16, kind="ExternalInput").ap()
    out = nc.dram_tensor("out", [NPC, H * DH], F32, kind="ExternalOutput").ap()

    with tile.TileContext(nc) as tc, ExitStack() as ctx:
        cpool = ctx.enter_context(tc.tile_pool(name="c", bufs=1))
        gpool = ctx.enter_context(tc.tile_pool(name="g", bufs=4))
        wpool = ctx.enter_context(tc.tile_pool(name="wk", bufs=3))
        opool = ctx.enter_context(tc.tile_pool(name="o", bufs=2))
        pfeat = ctx.enter_context(tc.tile_pool(name="pf", bufs=2, space="PSUM"))

        idx_sb = cpool.tile([128, npad // 16], I16, tag="idx", name="idx")
        nc.sync.dma_start(idx_sb[:], idx_in[:])
        dc_sb = cpool.tile([128, BPC * s_max], BF16, tag="dc", name="dc")
        nc.sync.dma_start(dc_sb[:], dcol_c[:])
        alf_sb = cpool.tile([128, BPC, s_max, H], BF16, tag="alf", name="alf")
        nc.sync.dma_start(
            alf_sb[:], alf_in.rearrange("p (b s h) -> p b s h", b=BPC, s=s_max))
        ior_sb = cpool.tile([128, 128], BF16, tag="ior", name="ior")
        nc.sync.dma_start(ior_sb[:], iota_r[:])

        gdma_sem = nc.alloc_semaphore("gdma")

        for b in range(BPC):
            # async gather: desc-gen on Pool, transfer fired by trigger_dma;
            # Tile attributes the g_sb write to the prep's DMA completion.
            g_sb = gpool.tile([128, s_max, ELEM], GDT, tag="gath", name="gath")
            nc.gpsimd.dma_gather(
                g_sb[:], table[:],
                idx_sb[:, b * (p_b // 16):(b + 1) * (p_b // 16)],
                p_b, p_b, ELEM, single_packet=False,
                prepare_only=True, sem=gdma_sem)
            nc.gpsimd.trigger_dma(count=None)

            # dst one-hot masks for the whole block: m[e, s, d] = (dcol==d)
            m_sb = wpool.tile([128, s_max, 128], BF16, tag="m", name="m")
            dcs = dc_sb[:, b * s_max:(b + 1) * s_max]
            nc.vector.tensor_tensor(
                m_sb[:],
                dcs.unsqueeze(2).to_broadcast((128, s_max, 128)),
                ior_sb[:].unsqueeze(1).to_broadcast((128, s_max, 128)),
                op=mybir.AluOpType.is_equal)

            # alpha-weighted gathered rows (padded lanes have alpha=0).
            # Two-step: dense broadcast-expand at 4x copy rate, then a
            # dense 2x tensor_tensor — faster than one broadcast multiply
            # which drops DVE to the 1x tier.
            wt_sb = wpool.tile([128, s_max, H, DH], BF16, tag="wt", name="wt")
            nc.vector.tensor_copy(
                wt_sb[:],
                alf_sb[:, b, :, :].unsqueeze(3).to_broadcast((128, s_max, H, DH)))
            nc.vector.tensor_tensor(
                wt_sb[:], wt_sb[:],
                g_sb[:].rearrange("p s (h d) -> p s h d", d=DH),
                op=mybir.AluOpType.mult)

            # segment-sum via masked matmul accumulating in PSUM
            pf = pfeat.tile([128, H * DH], F32, tag="pf", name="pf")
            for s in range(s_max):
                nc.tensor.matmul(
                    pf[:],
                    m_sb[:, s, :],
                    wt_sb[:, s, :, :],
                    start=(s == 0), stop=(s == s_max - 1))

            # epilogue on the idle Scalar engine (bias folded into phase A)
            o_sb = opool.tile([128, H * DH], F32, tag="osb", name="osb")
            nc.scalar.activation(o_sb[:], pf[:],
                                 mybir.ActivationFunctionType.Copy)
            nc.sync.dma_start(out[b * 128:(b + 1) * 128, :], o_sb[:])
    nc.compile()
    return nc


# ----------------------------------------------------------------------------
# Host side
# ----------------------------------------------------------------------------

def _preprocess(src, dst):
    deg = np.bincount(dst, minlength=N)
    order = np.argsort(-deg, kind="stable")
    ranks = np.arange(N)
    rounds, pos = ranks // NBLK, ranks % NBLK
    blk = np.where(rounds % 2 == 0, pos, NBLK - 1 - pos)
    new_id = np.empty(N, np.int64)
    new_id[order] = blk * 128 + rounds
    bsum = np.bincount(new_id[dst] // 128, minlength=NBLK)
    s_max = int(np.ceil(bsum.max() / 128))
    p_b = s_max * 128
    s2, d2 = new_id[src], new_id[dst]
    eo = np.argsort(d2, kind="stable")
    s2, d2 = s2[eo], d2[eo]
    starts = np.concatenate([[0], np.cumsum(bsum)])
    eblk = d2 // 128
    flatpos = eblk * p_b + (np.arange(len(d2)) - starts[eblk])
    bsrc = np.zeros(NBLK * p_b, np.int16)
    bsrc[flatpos] = s2.astype(np.int16)
    bcol = np.full(NBLK * p_b, 255.0, np.float32)
    bcol[flatpos] = (d2 % 128).astype(np.float32)
    bsrc = bsrc.reshape(NBLK, p_b)
    bcol = bcol.reshape(NBLK, p_b)
    return new_id, bsrc, bcol, s_max, s2, d2, flatpos


_CACHE = {}


class _Runner:
    """Cached SPMD runner: jits the bass_exec body once per Bass module."""

    def __init__(self, nc):
        install_neuronx_cc_hook()
        self.nc = nc
        part_name = (nc.partition_id_tensor.name
                     if nc.partition_id_tensor else None)
        in_names, out_names, out_avals, zero_outs = [], [], [], []
        for alloc in nc.m.functions[0].allocations:
            if not isinstance(alloc, mybir.MemoryLocationSet):
                continue
            name = alloc.memorylocations[0].name
            if alloc.kind == "ExternalInput":
                if name != part_name:
                    in_names.append(name)
            elif alloc.kind == "ExternalOutput":
                out_names.append(name)
                shape = tuple(alloc.tensor_shape)
                dtype = mybir.dt.np(alloc.dtype)
                out_avals.append(jax.core.ShapedArray(shape, dtype))
                zero_outs.append(np.zeros(shape, dtype))
        self.in_names, self.out_names = in_names, out_names
        self.out_avals, self.zero_outs = out_avals, zero_outs
        n_params, n_outs = len(in_names), len(out_avals)
        all_names = tuple(in_names + out_names
                          + ([part_name] if part_name else []))
        avals = tuple(out_avals)

        def _body(*args):
            operands = list(args)
            if part_name is not None:
                operands.append(partition_id_tensor())
            outs = _bass_exec_p.bind(
                *operands,
                out_avals=avals,
                in_names=all_names,
                out_names=tuple(out_names),
                lowering_input_output_aliases=(),
                sim_require_finite=True,
                sim_require_nnan=True,
                nc=nc,
            )
            return tuple(outs)

        devices = jax.devices()[:NC]
        self.mesh = Mesh(np.asarray(devices), ("core",))
        in_specs = (PartitionSpec("core"),) * (n_params + n_outs)
        out_specs = (PartitionSpec("core"),) * n_outs
        self.fn = jax.jit(
            shard_map(_body, mesh=self.mesh, in_specs=in_specs,
                      out_specs=out_specs, check_rep=False),
            keep_unused=True)

    def prep(self, in_maps):
        """Concatenate per-core inputs along axis 0 (host)."""
        n_params = len(self.in_names)
        concat_in = [
            np.concatenate([in_maps[c][self.in_names[i]] for c in range(NC)],
                           axis=0)
            for i in range(n_params)]
        concat_zeros = [
            np.zeros((NC * z.shape[0], *z.shape[1:]), z.dtype)
            for z in self.zero_outs]
        return concat_in + concat_zeros

    def run_prepped(self, args):
        return self.fn(*args)

    def run(self, in_maps):
        out_arrs = self.fn(*self.prep(in_maps))
        return [
            {name: np.asarray(out_arrs[i]).reshape(NC, *self.out_avals[i].shape)[c]
             for i, name in enumerate(self.out_names)}
            for c in range(NC)]


def _get_kernels(s_max):
    key = (s_max, MM_F32R)
    if key not in _CACHE:
        _CACHE[key] = (_Runner(build_phase_a()), _Runner(build_phase_b(s_max)))
    return _CACHE[key]


def kernel(text, weight, fc_w, attn_l, attn_r, bias, src, dst):
    text = np.asarray(text, np.float32)
    weight = np.asarray(weight, np.float32)
    fc_w = np.asarray(fc_w, np.float32)
    attn_l = np.asarray(attn_l, np.float32)
    attn_r = np.asarray(attn_r, np.float32)
    bias = np.asarray(bias, np.float32)
    src = np.asarray(src).astype(np.int64)
    dst = np.asarray(dst).astype(np.int64)

    new_id, bsrc, bcol, s_max, s2, d2, flatpos = _preprocess(src, dst)
    p_b = s_max * 128
    npad = BPC * p_b
    orig_for_new = np.empty(N, np.int64)
    orig_for_new[new_id] = np.arange(N)

    run_a, run_b = _get_kernels(s_max)

    # --- launch A ---
    attn_cat = np.zeros((DIN, 2 * H), np.float32)
    for h in range(H):
        attn_cat[h * DH:(h + 1) * DH, h] = attn_l[h]
        attn_cat[h * DH:(h + 1) * DH, H + h] = attn_r[h]
    ident = np.eye(128, dtype=np.float32)
    text_flat = text.reshape(N, DIN)
    in_maps_a = []
    bias_col = np.ascontiguousarray(bias.reshape(DIN, 1))
    for c in range(NC):
        rows = orig_for_new[c * NPC:(c + 1) * NPC]
        textT = np.ascontiguousarray(text_flat[rows].T)
        in_maps_a.append({"textT": textT, "weight": weight, "fc_w": fc_w,
                          "attn": attn_cat, "ident": ident, "biasv": bias_col})
    res_a = run_a.run(in_maps_a)

    table_full = np.concatenate([r["table"] for r in res_a], axis=0)
    elr_full = np.concatenate([r["elr"] for r in res_a], axis=1)  # [8, N]
    el_full = elr_full[:H].T.astype(np.float32)    # [N, H] (new-id order)
    er_full = elr_full[H:].T.astype(np.float32)

    # --- host softmax over edges (sorted by dst) ---
    e_log = el_full[s2] + er_full[d2]                       # [E, H]
    e_log = np.where(e_log > 0, e_log, NEG * e_log)
    seg = np.searchsorted(d2, np.arange(N))                 # segment starts
    emax = np.maximum.reduceat(e_log, seg, axis=0)          # [N, H]
    ex = np.exp(e_log - emax[d2])
    den = np.add.reduceat(ex, seg, axis=0)
    alpha = (ex / den[d2]).astype(np.float32)               # [E, H]
    alf_pad = np.zeros((NBLK * p_b, H), np.float32)
    alf_pad[flatpos] = alpha
    alf_pad = alf_pad.reshape(NBLK, s_max, 128, H)

    # --- launch B ---
    iota_row = np.broadcast_to(
        np.arange(128, dtype=np.float32), (128, 128)).astype(ml_dtypes.bfloat16)
    in_maps_b = []
    for c in range(NC):
        blks = range(c * BPC, (c + 1) * BPC)
        idx16 = np.concatenate(
            [bsrc[b].reshape(p_b // 16, 16).T for b in blks], axis=1)
        idx16 = np.ascontiguousarray(np.tile(idx16, (8, 1)))
        dcolc = np.concatenate(
            [bcol[b].reshape(s_max, 128).T for b in blks], axis=1)
        dcolc = np.ascontiguousarray(dcolc).astype(ml_dtypes.bfloat16)
        alf = np.ascontiguousarray(
            alf_pad[c * BPC:(c + 1) * BPC].transpose(2, 0, 1, 3).reshape(
                128, BPC * s_max * H)).astype(ml_dtypes.bfloat16)
        in_maps_b.append({
            "table": table_full, "idx16": idx16, "dcolc": dcolc,
            "alf": alf, "iotar": iota_row})
    res_b = run_b.run(in_maps_b)

    out_new = np.concatenate([r["out"] for r in res_b], axis=0)
    result = out_new[new_id].reshape(B, L, H * DH).astype(np.float32)

    global _LAST_ARGS
    _LAST_ARGS = (run_a, in_maps_a, run_b, in_maps_b)
    return result


_LAST_ARGS = None


# revision 20
# speedup vs baseline: 1.5442x; 1.5442x over previous
"""Trainium2 Bass kernel for nn_DglGraphAttentionNetwork (GAT layer over a
random graph, B=16, L=1024, DIN=512, H=4 heads, DH=128).

Strategy (8 NeuronCores, SPMD):
  Launch A (data-parallel over nodes): each core projects its 2048 nodes
    (hidden = text@W, h = hidden@fc_w, el/er attention dots) and emits the
    h rows as a bf16 gather table [2048, 512] plus el/er [8, 2048].
  Host: concatenates the 8 table slices into the full [16384, 512] table
    and computes the per-edge softmax weights alpha from el/er (cheap
    scalar pipeline over the edge list; edges are pre-sorted by dst).
  Launch B (dst-sharded): edges are sorted by destination and grouped into
    128-destination blocks (host balances blocks by relabeling nodes). Each
    core handles 16 blocks: h[src] rows are fetched with indirect dma_gather
    (128 edges across partitions), scaled by the uploaded alpha, and the
    per-destination segment-sum is a masked matmul accumulating in PSUM.
"""

import os
import sys

sys.path.insert(0, "/opt/trn_rl_repo")

from contextlib import ExitStack

import numpy as np
import ml_dtypes

import jax
from jax.sharding import Mesh, PartitionSpec
from jax.experimental.shard_map import shard_map

try:
    jax.config.update("jax_compilation_cache_dir", "/tmp/gat_jax_cache")
    jax.config.update("jax_persistent_cache_min_compile_time_secs", 1.0)
    jax.config.update("jax_persistent_cache_min_entry_size_bytes", -1)
except Exception:
    pass

import concourse.bass as bass
import concourse.bacc as bacc
import concourse.mybir as mybir
import concourse.tile as tile
from concourse import library_config
from concourse import bass2jax
from concourse.bass2jax import _bass_exec_p, install_neuronx_cc_hook, partition_id_tensor

F32 = mybir.dt.float32
F32R = mybir.dt.float32r
BF16 = mybir.dt.bfloat16
I16 = mybir.dt.int16

B, L, DIN = 16, 1024, 512
H, DH = 4, 128
N = B * L           # 16384 nodes
NC = 8              # cores
NPC = N // NC       # 2048 nodes per core
NBLK = 128          # destination blocks of 128 nodes
BPC = NBLK // NC    # 16 blocks per core
NEG = 0.2           # leaky_relu slope
ELEM = 512          # gather-table row: just the h features (bf16 -> 1024B)

# The SWDGE gather descriptor generation on the Pool/Q7 engine runs at only
# ~9 ns/descriptor and is phase B's hard bottleneck. Split it between the two
# launches: phase A (whose Pool engine is idle) pre-gathers, for each dst
# block, SA subtiles' worth of edges whose src falls in the core's own node
# range (from its local table slice), and writes them as a contiguous
# edge-ordered payload. Phase B streams that payload with one cheap
# contiguous DMA per block and only runs desc-gen for the remaining SB
# subtiles.
SA = 7              # payload (phase-A pre-gathered) subtiles per block
QA = SA * 128       # 896 A-side edges per block
QAC = QA // NC      # 112 per (block, src-core) cell
NAP = NBLK * QAC    # 14336 payload rows per phase-A core
ACH = 2048          # phase-A gather chunk (rows per dma_gather)

GDT = BF16
GNP = ml_dtypes.bfloat16
# phase-A matmul dtype: float32r streams at bf16 rate with fp32 operands,
# but requires f32r-rounding producers (see build_phase_a).
MM_F32R = os.environ.get("GAT_MMDT", "f32r") == "f32r"
MMDT = F32R if MM_F32R else F32


# ----------------------------------------------------------------------------
# Launch A: projection. Per core: textT [512, 2048] -> table [2048, 512], elr.
# ----------------------------------------------------------------------------

def build_phase_a():
    nc = bacc.Bacc("TRN2", target_bir_lowering=False, debug=False,
                   enable_asserts=False, num_devices=NC)
    textT = nc.dram_tensor("textT", [DIN, NPC], F32, kind="ExternalInput").ap()
    weight = nc.dram_tensor("weight", [DIN, DIN], F32, kind="ExternalInput").ap()
    fc_w = nc.dram_tensor("fc_w", [DIN, DIN], F32, kind="ExternalInput").ap()
    attn = nc.dram_tensor("attn", [DIN, 2 * H], F32, kind="ExternalInput").ap()
    ident = nc.dram_tensor("ident", [128, 128], F32, kind="ExternalInput").ap()
    bias_in = nc.dram_tensor("biasv", [DIN, 1], F32, kind="ExternalInput").ap()
    gidx_in = nc.dram_tensor("gidx", [128, NAP // 16], I16, kind="ExternalInput").ap()
    table = nc.dram_tensor("table", [NPC, ELEM], GDT, kind="ExternalOutput").ap()
    elr = nc.dram_tensor("elr", [2 * H, NPC], F32, kind="ExternalOutput").ap()
    pay = nc.dram_tensor("pay", [NAP, ELEM], GDT, kind="ExternalOutput").ap()

    KT = DIN // 128  # 4 contraction tiles

    with tile.TileContext(nc) as tc, ExitStack() as ctx:
        wpool = ctx.enter_context(tc.tile_pool(name="w", bufs=1))
        hpool = ctx.enter_context(tc.tile_pool(name="h", bufs=1))
        tpool = ctx.enter_context(tc.tile_pool(name="t", bufs=3))
        gpool = ctx.enter_context(tc.tile_pool(name="ga", bufs=2))
        pmm = ctx.enter_context(tc.tile_pool(name="pmm", bufs=2, space="PSUM"))
        pelr = ctx.enter_context(tc.tile_pool(name="pelr", bufs=2, space="PSUM"))
        ptr = ctx.enter_context(tc.tile_pool(name="ptr", bufs=2, space="PSUM"))

        # Load via DMA then launder through one DVE copy each: fp32 matmuls
        # carry a single sync-wait slot in codegen, so every matmul operand
        # must be produced by the same engine (DVE) rather than by one of
        # the 8 round-robin DMA semaphore lanes.
        w_ld = [wpool.tile([128, DIN], F32, tag=f"wl{i}", name=f"wl{i}") for i in range(KT)]
        fc_ld = [wpool.tile([128, DIN], F32, tag=f"fl{i}", name=f"fl{i}") for i in range(KT)]
        attn_ld = [wpool.tile([128, 2 * H], F32, tag=f"al{i}", name=f"al{i}") for i in range(KT)]
        tT_ld = [wpool.tile([128, NPC], F32, tag=f"tl{i}", name=f"tl{i}") for i in range(KT)]
        id_ld = wpool.tile([128, 128], F32, tag="idl", name="idl")
        for i in range(KT):
            nc.sync.dma_start(w_ld[i][:], weight[i * 128:(i + 1) * 128, :])
            nc.sync.dma_start(fc_ld[i][:], fc_w[i * 128:(i + 1) * 128, :])
            nc.sync.dma_start(attn_ld[i][:], attn[i * 128:(i + 1) * 128, :])
            nc.sync.dma_start(tT_ld[i][:], textT[i * 128:(i + 1) * 128, :])
        nc.sync.dma_start(id_ld[:], ident[:])
        # bias[f] laid out feature-on-partition: col t holds bias[t*128+p]
        bias_sb = wpool.tile([128, KT], F32, tag="bv", name="bv")
        nc.sync.dma_start(bias_sb[:], bias_in.rearrange("(t p) o -> p (t o)", p=128))
        w_sb = [wpool.tile([128, DIN], MMDT, tag=f"w{i}", name=f"w{i}") for i in range(KT)]
        fc_sb = [wpool.tile([128, DIN], MMDT, tag=f"fc{i}", name=f"fc{i}") for i in range(KT)]
        attn_sb = [wpool.tile([128, 2 * H], MMDT, tag=f"at{i}", name=f"at{i}") for i in range(KT)]
        tT_sb = [wpool.tile([128, NPC], MMDT, tag=f"tt{i}", name=f"tt{i}") for i in range(KT)]
        id_sb = wpool.tile([128, 128], MMDT, tag="id", name="id")
        for i in range(KT):
            nc.vector.tensor_copy(w_sb[i][:], w_ld[i][:])
            nc.vector.tensor_copy(fc_sb[i][:], fc_ld[i][:])
            nc.vector.tensor_copy(attn_sb[i][:], attn_ld[i][:])
            nc.vector.tensor_copy(tT_sb[i][:], tT_ld[i][:])
        nc.vector.tensor_copy(id_sb[:], id_ld[:])

        # hiddenT[e, n] = sum_d W[d, e] * textT[d, n]
        h1_sb = [hpool.tile([128, NPC], MMDT, tag=f"h1{i}", name=f"h1{i}") for i in range(KT)]
        for et in range(KT):
            for nch in range(NPC // 512):
                p = pmm.tile([128, 512], F32, tag="pmm", name="pmm")
                for dt in range(KT):
                    nc.tensor.matmul(
                        p[:],
                        w_sb[dt][:, et * 128:(et + 1) * 128],
                        tT_sb[dt][:, nch * 512:(nch + 1) * 512],
                        start=(dt == 0), stop=(dt == KT - 1))
                nc.vector.tensor_copy(h1_sb[et][:, nch * 512:(nch + 1) * 512], p[:])

        # hT[f, n] = sum_e fc_w[e, f] * hiddenT[e, n]
        h2_sb = [hpool.tile([128, NPC], MMDT, tag=f"h2{i}", name=f"h2{i}") for i in range(KT)]
        for ft in range(KT):
            for nch in range(NPC // 512):
                p = pmm.tile([128, 512], F32, tag="pmm", name="pmm")
                for et in range(KT):
                    nc.tensor.matmul(
                        p[:],
                        fc_sb[et][:, ft * 128:(ft + 1) * 128],
                        h1_sb[et][:, nch * 512:(nch + 1) * 512],
                        start=(et == 0), stop=(et == KT - 1))
                # + bias here: out = sum_e alpha_e (h[src]+bias) = agg + bias
                # since softmax weights sum to 1; the el/er shift it also
                # induces is constant per head, which softmax cancels.
                nc.vector.tensor_scalar(
                    h2_sb[ft][:, nch * 512:(nch + 1) * 512], p[:],
                    bias_sb[:, ft:ft + 1], None, op0=mybir.AluOpType.add)

        # elrT[c, n] (c = 4 el heads then 4 er heads)
        elr_sb = hpool.tile([2 * H, NPC], F32, tag="elr", name="elr")
        for nch in range(NPC // 512):
            p = pelr.tile([2 * H, 512], F32, tag="pelr", name="pelr")
            for ft in range(KT):
                nc.tensor.matmul(
                    p[:],
                    attn_sb[ft][:],
                    h2_sb[ft][:, nch * 512:(nch + 1) * 512],
                    start=(ft == 0), stop=(ft == KT - 1))
            nc.vector.tensor_copy(elr_sb[:, nch * 512:(nch + 1) * 512], p[:])
        nc.sync.dma_start(elr[:], elr_sb[:])

        # per 128-node tile: transpose hT into row-major table rows
        for nt in range(NPC // 128):
            tab = tpool.tile([128, ELEM], GDT, tag="tab", name="tab")
            for ft in range(KT):
                pt = ptr.tile([128, 128], MMDT, tag="ptr", name="ptr")
                nc.tensor.transpose(
                    pt[:], h2_sb[ft][:, nt * 128:(nt + 1) * 128], id_sb[:])
                nc.scalar.activation(
                    tab[:, ft * DH:(ft + 1) * DH], pt[:],
                    mybir.ActivationFunctionType.Copy)
            nc.sync.dma_start(table[nt * 128:(nt + 1) * 128, :], tab[:])

        # idx tile for the local payload gather
        gidx_sb = wpool.tile([128, NAP // 16], I16, tag="gix", name="gix")
        nc.sync.dma_start(gidx_sb[:], gidx_in[:])

        # pre-gather the A-side edge payload from the just-written local
        # table slice (this is the half of phase B's desc-gen that rides in
        # phase A's Pool-idle shadow)
        for ch in range(NAP // ACH):
            g_sb = gpool.tile([128, ACH // 128, ELEM], GDT, tag="gch",
                              name="gch")
            nc.gpsimd.dma_gather(
                g_sb[:], table[:],
                gidx_sb[:, ch * (ACH // 16):(ch + 1) * (ACH // 16)],
                ACH, ACH, ELEM, single_packet=False)
            nc.sync.dma_start(
                pay[ch * ACH:(ch + 1) * ACH, :].rearrange(
                    "(s p) e -> p s e", p=128),
                g_sb[:])
    nc.compile()
    return nc


# ----------------------------------------------------------------------------
# Launch B: weighted aggregation (alpha precomputed on host), dst-sharded.
# ----------------------------------------------------------------------------

def build_phase_b(sb: int):
    S = sb + SA                # total subtiles per block
    p_b = sb * 128             # B-side (gathered) lanes per block
    npad = BPC * p_b           # B-side lanes per core

    nc = bacc.Bacc("TRN2", target_bir_lowering=False, debug=False,
                   enable_asserts=False, num_devices=NC)
    table = nc.dram_tensor("table", [N, ELEM], GDT, kind="ExternalInput").ap()
    payb = nc.dram_tensor("payb", [BPC * QA, ELEM], GDT, kind="ExternalInput").ap()
    idx_in = nc.dram_tensor("idx16", [128, npad // 16], I16, kind="ExternalInput").ap()
    dcol_c = nc.dram_tensor("dcolc", [128, BPC * S], BF16, kind="ExternalInput").ap()
    alf_in = nc.dram_tensor("alf", [128, BPC * S * H], BF16, kind="ExternalInput").ap()
    iota_r = nc.dram_tensor("iotar", [128, 128], BF16, kind="ExternalInput").ap()
    out = nc.dram_tensor("out", [NPC, H * DH], F32, kind="ExternalOutput").ap()

    with tile.TileContext(nc) as tc, ExitStack() as ctx:
        cpool = ctx.enter_context(tc.tile_pool(name="c", bufs=1))
        gpool = ctx.enter_context(tc.tile_pool(name="g", bufs=3))
        wpool = ctx.enter_context(tc.tile_pool(name="wk", bufs=3))
        opool = ctx.enter_context(tc.tile_pool(name="o", bufs=2))
        pfeat = ctx.enter_context(tc.tile_pool(name="pf", bufs=2, space="PSUM"))

        idx_sb = cpool.tile([128, npad // 16], I16, tag="idx", name="idx")
        nc.sync.dma_start(idx_sb[:], idx_in[:])
        dc_sb = cpool.tile([128, BPC * S], BF16, tag="dc", name="dc")
        nc.sync.dma_start(dc_sb[:], dcol_c[:])
        alf_sb = cpool.tile([128, BPC, S, H], BF16, tag="alf", name="alf")
        nc.sync.dma_start(
            alf_sb[:], alf_in.rearrange("p (b s h) -> p b s h", b=BPC, s=S))
        ior_sb = cpool.tile([128, 128], BF16, tag="ior", name="ior")
        nc.sync.dma_start(ior_sb[:], iota_r[:])

        for b in range(BPC):
            g_sb = gpool.tile([128, S, ELEM], GDT, tag="gath", name="gath")
            # B-side lanes: indexed gather (Pool desc-gen)
            nc.gpsimd.dma_gather(
                g_sb[:, 0:sb, :], table[:],
                idx_sb[:, b * (p_b // 16):(b + 1) * (p_b // 16)],
                p_b, p_b, ELEM, single_packet=False)
            # A-side lanes: contiguous payload stream (HWDGE, no desc-gen)
            nc.sync.dma_start(
                g_sb[:, sb:S, :],
                payb[b * QA:(b + 1) * QA, :].rearrange(
                    "(s p) e -> p s e", p=128))

            # dst one-hot masks for the whole block: m[e, s, d] = (dcol==d)
            m_sb = wpool.tile([128, S, 128], BF16, tag="m", name="m")
            dcs = dc_sb[:, b * S:(b + 1) * S]
            nc.vector.tensor_tensor(
                m_sb[:],
                dcs.unsqueeze(2).to_broadcast((128, S, 128)),
                ior_sb[:].unsqueeze(1).to_broadcast((128, S, 128)),
                op=mybir.AluOpType.is_equal)

            # alpha-weighted gathered rows (padded lanes have alpha=0).
            # Broadcast-expand at the 2x copy tier then dense in-place
            # multiply (fast tier) — a single broadcast multiply drops the
            # DVE to its 1x tier and is ~30% slower overall.
            wt_sb = wpool.tile([128, S, H, DH], BF16, tag="wt", name="wt")
            nc.vector.tensor_copy(
                wt_sb[:],
                alf_sb[:, b, :, :].unsqueeze(3).to_broadcast((128, S, H, DH)))
            nc.vector.tensor_tensor(
                wt_sb[:], wt_sb[:],
                g_sb[:].rearrange("p s (h d) -> p s h d", d=DH),
                op=mybir.AluOpType.mult)

            # segment-sum via masked matmul accumulating in PSUM
            pf = pfeat.tile([128, H * DH], F32, tag="pf", name="pf")
            for s in range(S):
                nc.tensor.matmul(
                    pf[:],
                    m_sb[:, s, :],
                    wt_sb[:, s, :, :],
                    start=(s == 0), stop=(s == S - 1))

            # epilogue on the idle Scalar engine (bias folded into phase A)
            o_sb = opool.tile([128, H * DH], F32, tag="osb", name="osb")
            nc.scalar.activation(o_sb[:], pf[:],
                                 mybir.ActivationFunctionType.Copy)
            nc.sync.dma_start(out[b * 128:(b + 1) * 128, :], o_sb[:])
    nc.compile()
    return nc


# ----------------------------------------------------------------------------
# Host side
# ----------------------------------------------------------------------------

def _assign_blocks(deg):
    """LPT assignment of nodes to 128-node blocks balancing in-degree sums,
    plus a swap refinement pass to pull the max block sum down to the mean
    (E/NBLK) when possible."""
    import heapq
    order = np.argsort(-deg, kind="stable")
    cap = N // NBLK
    hp = [(0, 0, b) for b in range(NBLK)]
    heapq.heapify(hp)
    asg = np.empty(N, np.int64)
    members = [[] for _ in range(NBLK)]
    for i in order:
        s, c, b = heapq.heappop(hp)
        asg[i] = b
        members[b].append(i)
        c += 1
        s += int(deg[i])
        if c < cap:
            heapq.heappush(hp, (s, c, b))
    bsum = np.zeros(NBLK, np.int64)
    for b in range(NBLK):
        bsum[b] = deg[members[b]].sum()
    target = int(np.ceil(deg.sum() / NBLK))
    for _ in range(512):
        hi = int(bsum.argmax())
        if bsum[hi] <= target:
            break
        lo = int(bsum.argmin())
        need = int(bsum[hi] - target)
        degs_lo = {int(deg[v]): v for v in members[lo]}
        done = False
        for u in members[hi]:
            du = int(deg[u])
            for t in range(min(need, 8), 0, -1):
                v = degs_lo.get(du - t)
                if v is not None and bsum[lo] + t <= target:
                    members[hi].remove(u)
                    members[lo].remove(v)
                    members[hi].append(v)
                    members[lo].append(u)
                    asg[u], asg[v] = lo, hi
                    bsum[hi] -= t
                    bsum[lo] += t
                    done = True
                    break
            if done:
                break
        if not done:
            break
    return asg, bsum


def _preprocess(src, dst):
    deg = np.bincount(dst, minlength=N)
    asg, bsum = _assign_blocks(deg)
    # position within block: arbitrary stable order
    new_id = np.empty(N, np.int64)
    eo_n = np.argsort(asg, kind="stable")
    new_id[eo_n] = np.arange(N)

    s2, d2 = new_id[src], new_id[dst]
    eo = np.argsort(d2, kind="stable")
    s2, d2 = s2[eo], d2[eo]
    E = len(s2)
    eblk = d2 // 128

    sb = int(np.ceil((bsum.max() - QA) / 128))
    p_b = sb * 128
    S = sb + SA

    # A-side selection: per (block, src-core) cell take the first QAC edges
    sc = s2 // NPC
    key = eblk * NC + sc
    ko = np.argsort(key, kind="stable")
    kstarts = np.searchsorted(key[ko], np.arange(NBLK * NC))
    rank = np.empty(E, np.int64)
    rank[ko] = np.arange(E) - kstarts[key[ko]]
    cell_counts = np.bincount(key, minlength=NBLK * NC)
    assert cell_counts.min() >= QAC, cell_counts.min()
    aside = rank < QAC

    # A-side: payload position (per src-core) and lane within block
    pay_pos = eblk * QAC + rank          # valid where aside
    lane = np.empty(E, np.int64)
    lane[aside] = p_b + sc[aside] * QAC + rank[aside]

    # B-side: rank among the block's B-side edges -> lanes [0, p_b)
    bkey = np.where(aside, -1, eblk)
    bo = np.argsort(bkey, kind="stable")
    nA = int(aside.sum())
    bstarts = np.searchsorted(bkey[bo], np.arange(NBLK))
    rankB = np.empty(E, np.int64)
    rankB[bo] = np.arange(E) - bstarts[bkey[bo]]
    assert rankB[~aside].max() < p_b
    lane[~aside] = rankB[~aside]

    flat = eblk * (S * 128) + lane

    bsrc = np.zeros((NBLK, p_b), np.int16)
    bsrc[eblk[~aside], lane[~aside]] = s2[~aside].astype(np.int16)
    bcol = np.full((NBLK, S * 128), 255.0, np.float32)
    bcol[eblk, lane] = (d2 % 128).astype(np.float32)

    return dict(new_id=new_id, s2=s2, d2=d2, sb=sb, S=S, p_b=p_b,
                bsrc=bsrc, bcol=bcol, flat=flat, aside=aside, sc=sc,
                pay_pos=pay_pos)


_CACHE = {}


class _Runner:
    """Cached SPMD runner: jits the bass_exec body once per Bass module."""

    def __init__(self, nc):
        install_neuronx_cc_hook()
        self.nc = nc
        part_name = (nc.partition_id_tensor.name
                     if nc.partition_id_tensor else None)
        in_names, out_names, out_avals, zero_outs = [], [], [], []
        for alloc in nc.m.functions[0].allocations:
            if not isinstance(alloc, mybir.MemoryLocationSet):
                continue
            name = alloc.memorylocations[0].name
            if alloc.kind == "ExternalInput":
                if name != part_name:
                    in_names.append(name)
            elif alloc.kind == "ExternalOutput":
                out_names.append(name)
                shape = tuple(alloc.tensor_shape)
                dtype = mybir.dt.np(alloc.dtype)
                out_avals.append(jax.core.ShapedArray(shape, dtype))
                zero_outs.append(np.zeros(shape, dtype))
        self.in_names, self.out_names = in_names, out_names
        self.out_avals, self.zero_outs = out_avals, zero_outs
        n_params, n_outs = len(in_names), len(out_avals)
        all_names = tuple(in_names + out_names
                          + ([part_name] if part_name else []))
        avals = tuple(out_avals)

        def _body(*args):
            operands = list(args)
            if part_name is not None:
                operands.append(partition_id_tensor())
            outs = _bass_exec_p.bind(
                *operands,
                out_avals=avals,
                in_names=all_names,
                out_names=tuple(out_names),
                lowering_input_output_aliases=(),
                sim_require_finite=True,
                sim_require_nnan=True,
                nc=nc,
            )
            return tuple(outs)

        devices = jax.devices()[:NC]
        self.mesh = Mesh(np.asarray(devices), ("core",))
        in_specs = (PartitionSpec("core"),) * (n_params + n_outs)
        out_specs = (PartitionSpec("core"),) * n_outs
        self.fn = jax.jit(
            shard_map(_body, mesh=self.mesh, in_specs=in_specs,
                      out_specs=out_specs, check_rep=False),
            keep_unused=True)

    def prep(self, in_maps):
        """Concatenate per-core inputs along axis 0 (host)."""
        n_params = len(self.in_names)
        concat_in = [
            np.concatenate([in_maps[c][self.in_names[i]] for c in range(NC)],
                           axis=0)
            for i in range(n_params)]
        concat_zeros = [
            np.zeros((NC * z.shape[0], *z.shape[1:]), z.dtype)
            for z in self.zero_outs]
        return concat_in + concat_zeros

    def run_prepped(self, args):
        return self.fn(*args)

    def run(self, in_maps):
        out_arrs = self.fn(*self.prep(in_maps))
        return [
            {name: np.asarray(out_arrs[i]).reshape(NC, *self.out_avals[i].shape)[c]
             for i, name in enumerate(self.out_names)}
            for c in range(NC)]


def _get_kernels(sb):
    key = (sb, MM_F32R)
    if key not in _CACHE:
        _CACHE[key] = (_Runner(build_phase_a()), _Runner(build_phase_b(sb)))
    return _CACHE[key]


def kernel(text, weight, fc_w, attn_l, attn_r, bias, src, dst):
    text = np.asarray(text, np.float32)
    weight = np.asarray(weight, np.float32)
    fc_w = np.asarray(fc_w, np.float32)
    attn_l = np.asarray(attn_l, np.float32)
    attn_r = np.asarray(attn_r, np.float32)
    bias = np.asarray(bias, np.float32)
    src = np.asarray(src).astype(np.int64)
    dst = np.asarray(dst).astype(np.int64)

    pp = _preprocess(src, dst)
    new_id, s2, d2 = pp["new_id"], pp["s2"], pp["d2"]
    sb, S, p_b = pp["sb"], pp["S"], pp["p_b"]
    bsrc, bcol = pp["bsrc"], pp["bcol"]
    aside, sc, pay_pos = pp["aside"], pp["sc"], pp["pay_pos"]
    orig_for_new = np.empty(N, np.int64)
    orig_for_new[new_id] = np.arange(N)

    run_a, run_b = _get_kernels(sb)

    # --- launch A ---
    attn_cat = np.zeros((DIN, 2 * H), np.float32)
    for h in range(H):
        attn_cat[h * DH:(h + 1) * DH, h] = attn_l[h]
        attn_cat[h * DH:(h + 1) * DH, H + h] = attn_r[h]
    ident = np.eye(128, dtype=np.float32)
    text_flat = text.reshape(N, DIN)
    bias_col = np.ascontiguousarray(bias.reshape(DIN, 1))
    # per-src-core local gather indices in payload order
    gidx_all = np.zeros((NC, NAP), np.int16)
    gidx_all[sc[aside], pay_pos[aside]] = (s2[aside] % NPC).astype(np.int16)
    in_maps_a = []
    for c in range(NC):
        rows = orig_for_new[c * NPC:(c + 1) * NPC]
        textT = np.ascontiguousarray(text_flat[rows].T)
        gp = gidx_all[c].reshape(NAP // 16, 16).T
        gp = np.ascontiguousarray(np.tile(gp, (8, 1)))
        in_maps_a.append({"textT": textT, "weight": weight, "fc_w": fc_w,
                          "attn": attn_cat, "ident": ident, "biasv": bias_col,
                          "gidx": gp})
    res_a = run_a.run(in_maps_a)

    table_full = np.concatenate([r["table"] for r in res_a], axis=0)
    elr_full = np.concatenate([r["elr"] for r in res_a], axis=1)  # [8, N]
    el_full = elr_full[:H].T.astype(np.float32)    # [N, H] (new-id order)
    er_full = elr_full[H:].T.astype(np.float32)
    # payload reassembly: [core, blk, QAC, E] -> per dst-core block-major
    pay_all = np.stack([r["pay"] for r in res_a]).reshape(
        NC, NBLK, QAC, ELEM).transpose(1, 0, 2, 3)   # [NBLK, NC, QAC, ELEM]

    # --- host softmax over edges (sorted by dst) ---
    e_log = el_full[s2] + er_full[d2]                       # [E, H]
    e_log = np.where(e_log > 0, e_log, NEG * e_log)
    seg = np.searchsorted(d2, np.arange(N))                 # segment starts
    emax = np.maximum.reduceat(e_log, seg, axis=0)          # [N, H]
    ex = np.exp(e_log - emax[d2])
    den = np.add.reduceat(ex, seg, axis=0)
    alpha = (ex / den[d2]).astype(np.float32)               # [E, H]
    alf_pad = np.zeros((NBLK * S * 128, H), np.float32)
    alf_pad[pp["flat"]] = alpha
    alf_pad = alf_pad.reshape(NBLK, S, 128, H)

    # --- launch B ---
    iota_row = np.broadcast_to(
        np.arange(128, dtype=np.float32), (128, 128)).astype(ml_dtypes.bfloat16)
    in_maps_b = []
    for c in range(NC):
        blks = range(c * BPC, (c + 1) * BPC)
        idx16 = np.concatenate(
            [bsrc[b].reshape(p_b // 16, 16).T for b in blks], axis=1)
        idx16 = np.ascontiguousarray(np.tile(idx16, (8, 1)))
        dcolc = np.concatenate(
            [bcol[b].reshape(S, 128).T for b in blks], axis=1)
        dcolc = np.ascontiguousarray(dcolc).astype(ml_dtypes.bfloat16)
        alf = np.ascontiguousarray(
            alf_pad[c * BPC:(c + 1) * BPC].transpose(2, 0, 1, 3).reshape(
                128, BPC * S * H)).astype(ml_dtypes.bfloat16)
        payb = np.ascontiguousarray(
            pay_all[c * BPC:(c + 1) * BPC].reshape(BPC * QA, ELEM))
        in_maps_b.append({
            "table": table_full, "payb": payb, "idx16": idx16,
            "dcolc": dcolc, "alf": alf, "iotar": iota_row})
    res_b = run_b.run(in_maps_b)

    out_new = np.concatenate([r["out"] for r in res_b], axis=0)
    result = out_new[new_id].reshape(B, L, H * DH).astype(np.float32)

    global _LAST_ARGS
    _LAST_ARGS = (run_a, in_maps_a, run_b, in_maps_b)
    return result


_LAST_ARGS = None


# revision 22
# speedup vs baseline: 1.6687x; 1.0806x over previous
"""Trainium2 Bass kernel for nn_DglGraphAttentionNetwork (GAT layer over a
random graph, B=16, L=1024, DIN=512, H=4 heads, DH=128).

Strategy (8 NeuronCores, SPMD):
  Launch A (data-parallel over nodes): each core projects its 2048 nodes
    (hidden = text@W, h = hidden@fc_w, el/er attention dots) and emits the
    h rows as a bf16 gather table [2048, 512] plus el/er [8, 2048].
  Host: concatenates the 8 table slices into the full [16384, 512] table
    and computes the per-edge softmax weights alpha from el/er (cheap
    scalar pipeline over the edge list; edges are pre-sorted by dst).
  Launch B (dst-sharded): edges are sorted by destination and grouped into
    128-destination blocks (host balances blocks by relabeling nodes). Each
    core handles 16 blocks: h[src] rows are fetched with indirect dma_gather
    (128 edges across partitions), scaled by the uploaded alpha, and the
    per-destination segment-sum is a masked matmul accumulating in PSUM.
"""

import os
import sys

sys.path.insert(0, "/opt/trn_rl_repo")

from contextlib import ExitStack

import numpy as np
import ml_dtypes

import jax
from jax.sharding import Mesh, PartitionSpec
from jax.experimental.shard_map import shard_map

try:
    jax.config.update("jax_compilation_cache_dir", "/tmp/gat_jax_cache")
    jax.config.update("jax_persistent_cache_min_compile_time_secs", 1.0)
    jax.config.update("jax_persistent_cache_min_entry_size_bytes", -1)
except Exception:
    pass

import concourse.bass as bass
import concourse.bacc as bacc
import concourse.mybir as mybir
import concourse.tile as tile
from concourse import library_config
from concourse import bass2jax
from concourse.bass2jax import _bass_exec_p, install_neuronx_cc_hook, partition_id_tensor

F32 = mybir.dt.float32
F32R = mybir.dt.float32r
BF16 = mybir.dt.bfloat16
I16 = mybir.dt.int16

B, L, DIN = 16, 1024, 512
H, DH = 4, 128
N = B * L           # 16384 nodes
NC = 8              # cores
NPC = N // NC       # 2048 nodes per core
NBLK = 128          # destination blocks of 128 nodes
BPC = NBLK // NC    # 16 blocks per core
NEG = 0.2           # leaky_relu slope
ELEM = 512          # gather-table row: just the h features (bf16 -> 1024B)

# The SWDGE gather descriptor generation on the Pool/Q7 engine runs at only
# ~9 ns/descriptor and is phase B's hard bottleneck. Split it between the two
# launches: phase A (whose Pool engine is idle) pre-gathers, for each dst
# block, SA subtiles' worth of edges whose src falls in the core's own node
# range (from its local table slice), and writes them as a contiguous
# edge-ordered payload. Phase B streams that payload with one cheap
# contiguous DMA per block and only runs desc-gen for the remaining SB
# subtiles.
SA = 5              # payload (phase-A pre-gathered) subtiles per block
QA = SA * 128       # 896 A-side edges per block
QAC = QA // NC      # 112 per (block, src-core) cell
NAP = NBLK * QAC    # 14336 payload rows per phase-A core
ACH = 2048          # phase-A gather chunk (rows per dma_gather)

GDT = BF16
GNP = ml_dtypes.bfloat16
# phase-A matmul dtype: float32r streams at bf16 rate with fp32 operands,
# but requires f32r-rounding producers (see build_phase_a).
MM_F32R = os.environ.get("GAT_MMDT", "f32r") == "f32r"
MMDT = F32R if MM_F32R else F32


# ----------------------------------------------------------------------------
# Launch A: projection. Per core: textT [512, 2048] -> table [2048, 512], elr.
# ----------------------------------------------------------------------------

def build_phase_a():
    nc = bacc.Bacc("TRN2", target_bir_lowering=False, debug=False,
                   enable_asserts=False, num_devices=NC)
    textT = nc.dram_tensor("textT", [DIN, NPC], F32, kind="ExternalInput").ap()
    weight = nc.dram_tensor("weight", [DIN, DIN], F32, kind="ExternalInput").ap()
    fc_w = nc.dram_tensor("fc_w", [DIN, DIN], F32, kind="ExternalInput").ap()
    attn = nc.dram_tensor("attn", [DIN, 2 * H], F32, kind="ExternalInput").ap()
    ident = nc.dram_tensor("ident", [128, 128], F32, kind="ExternalInput").ap()
    bias_in = nc.dram_tensor("biasv", [DIN, 1], F32, kind="ExternalInput").ap()
    gidx_in = nc.dram_tensor("gidx", [128, NAP // 16], I16, kind="ExternalInput").ap()
    table = nc.dram_tensor("table", [NPC, ELEM], GDT, kind="ExternalOutput").ap()
    elr = nc.dram_tensor("elr", [2 * H, NPC], F32, kind="ExternalOutput").ap()
    pay = nc.dram_tensor("pay", [NAP, ELEM], GDT, kind="ExternalOutput").ap()

    KT = DIN // 128  # 4 contraction tiles

    with tile.TileContext(nc) as tc, ExitStack() as ctx:
        wpool = ctx.enter_context(tc.tile_pool(name="w", bufs=1))
        hpool = ctx.enter_context(tc.tile_pool(name="h", bufs=1))
        tpool = ctx.enter_context(tc.tile_pool(name="t", bufs=3))
        gpool = ctx.enter_context(tc.tile_pool(name="ga", bufs=2))
        pmm = ctx.enter_context(tc.tile_pool(name="pmm", bufs=2, space="PSUM"))
        pelr = ctx.enter_context(tc.tile_pool(name="pelr", bufs=2, space="PSUM"))
        ptr = ctx.enter_context(tc.tile_pool(name="ptr", bufs=2, space="PSUM"))

        # Load via DMA then launder through one DVE copy each: fp32 matmuls
        # carry a single sync-wait slot in codegen, so every matmul operand
        # must be produced by the same engine (DVE) rather than by one of
        # the 8 round-robin DMA semaphore lanes.
        w_ld = [wpool.tile([128, DIN], F32, tag=f"wl{i}", name=f"wl{i}") for i in range(KT)]
        fc_ld = [wpool.tile([128, DIN], F32, tag=f"fl{i}", name=f"fl{i}") for i in range(KT)]
        attn_ld = [wpool.tile([128, 2 * H], F32, tag=f"al{i}", name=f"al{i}") for i in range(KT)]
        tT_ld = [wpool.tile([128, NPC], F32, tag=f"tl{i}", name=f"tl{i}") for i in range(KT)]
        id_ld = wpool.tile([128, 128], F32, tag="idl", name="idl")
        for i in range(KT):
            nc.sync.dma_start(w_ld[i][:], weight[i * 128:(i + 1) * 128, :])
            nc.sync.dma_start(fc_ld[i][:], fc_w[i * 128:(i + 1) * 128, :])
            nc.sync.dma_start(attn_ld[i][:], attn[i * 128:(i + 1) * 128, :])
            nc.sync.dma_start(tT_ld[i][:], textT[i * 128:(i + 1) * 128, :])
        nc.sync.dma_start(id_ld[:], ident[:])
        # bias[f] laid out feature-on-partition: col t holds bias[t*128+p]
        bias_sb = wpool.tile([128, KT], F32, tag="bv", name="bv")
        nc.sync.dma_start(bias_sb[:], bias_in.rearrange("(t p) o -> p (t o)", p=128))
        w_sb = [wpool.tile([128, DIN], MMDT, tag=f"w{i}", name=f"w{i}") for i in range(KT)]
        fc_sb = [wpool.tile([128, DIN], MMDT, tag=f"fc{i}", name=f"fc{i}") for i in range(KT)]
        attn_sb = [wpool.tile([128, 2 * H], MMDT, tag=f"at{i}", name=f"at{i}") for i in range(KT)]
        tT_sb = [wpool.tile([128, NPC], MMDT, tag=f"tt{i}", name=f"tt{i}") for i in range(KT)]
        id_sb = wpool.tile([128, 128], MMDT, tag="id", name="id")
        for i in range(KT):
            nc.vector.tensor_copy(w_sb[i][:], w_ld[i][:])
            nc.vector.tensor_copy(fc_sb[i][:], fc_ld[i][:])
            nc.vector.tensor_copy(attn_sb[i][:], attn_ld[i][:])
            nc.vector.tensor_copy(tT_sb[i][:], tT_ld[i][:])
        nc.vector.tensor_copy(id_sb[:], id_ld[:])

        # hiddenT[e, n] = sum_d W[d, e] * textT[d, n]
        h1_sb = [hpool.tile([128, NPC], MMDT, tag=f"h1{i}", name=f"h1{i}") for i in range(KT)]
        for et in range(KT):
            for nch in range(NPC // 512):
                p = pmm.tile([128, 512], F32, tag="pmm", name="pmm")
                for dt in range(KT):
                    nc.tensor.matmul(
                        p[:],
                        w_sb[dt][:, et * 128:(et + 1) * 128],
                        tT_sb[dt][:, nch * 512:(nch + 1) * 512],
                        start=(dt == 0), stop=(dt == KT - 1))
                nc.vector.tensor_copy(h1_sb[et][:, nch * 512:(nch + 1) * 512], p[:])

        # hT[f, n] = sum_e fc_w[e, f] * hiddenT[e, n]
        h2_sb = [hpool.tile([128, NPC], MMDT, tag=f"h2{i}", name=f"h2{i}") for i in range(KT)]
        for ft in range(KT):
            for nch in range(NPC // 512):
                p = pmm.tile([128, 512], F32, tag="pmm", name="pmm")
                for et in range(KT):
                    nc.tensor.matmul(
                        p[:],
                        fc_sb[et][:, ft * 128:(ft + 1) * 128],
                        h1_sb[et][:, nch * 512:(nch + 1) * 512],
                        start=(et == 0), stop=(et == KT - 1))
                # + bias here: out = sum_e alpha_e (h[src]+bias) = agg + bias
                # since softmax weights sum to 1; the el/er shift it also
                # induces is constant per head, which softmax cancels.
                nc.vector.tensor_scalar(
                    h2_sb[ft][:, nch * 512:(nch + 1) * 512], p[:],
                    bias_sb[:, ft:ft + 1], None, op0=mybir.AluOpType.add)

        # elrT[c, n] (c = 4 el heads then 4 er heads)
        elr_sb = hpool.tile([2 * H, NPC], F32, tag="elr", name="elr")
        for nch in range(NPC // 512):
            p = pelr.tile([2 * H, 512], F32, tag="pelr", name="pelr")
            for ft in range(KT):
                nc.tensor.matmul(
                    p[:],
                    attn_sb[ft][:],
                    h2_sb[ft][:, nch * 512:(nch + 1) * 512],
                    start=(ft == 0), stop=(ft == KT - 1))
            nc.vector.tensor_copy(elr_sb[:, nch * 512:(nch + 1) * 512], p[:])
        nc.sync.dma_start(elr[:], elr_sb[:])

        # per 128-node tile: transpose hT into row-major table rows
        for nt in range(NPC // 128):
            tab = tpool.tile([128, ELEM], GDT, tag="tab", name="tab")
            for ft in range(KT):
                pt = ptr.tile([128, 128], MMDT, tag="ptr", name="ptr")
                nc.tensor.transpose(
                    pt[:], h2_sb[ft][:, nt * 128:(nt + 1) * 128], id_sb[:])
                nc.scalar.activation(
                    tab[:, ft * DH:(ft + 1) * DH], pt[:],
                    mybir.ActivationFunctionType.Copy)
            nc.sync.dma_start(table[nt * 128:(nt + 1) * 128, :], tab[:])

        # idx tile for the local payload gather
        gidx_sb = wpool.tile([128, NAP // 16], I16, tag="gix", name="gix")
        nc.sync.dma_start(gidx_sb[:], gidx_in[:])

        # pre-gather the A-side edge payload from the just-written local
        # table slice (this is the half of phase B's desc-gen that rides in
        # phase A's Pool-idle shadow)
        for ch in range(NAP // ACH):
            g_sb = gpool.tile([128, ACH // 128, ELEM], GDT, tag="gch",
                              name="gch")
            nc.gpsimd.dma_gather(
                g_sb[:], table[:],
                gidx_sb[:, ch * (ACH // 16):(ch + 1) * (ACH // 16)],
                ACH, ACH, ELEM, single_packet=False)
            nc.sync.dma_start(
                pay[ch * ACH:(ch + 1) * ACH, :].rearrange(
                    "(s p) e -> p s e", p=128),
                g_sb[:])
    nc.compile()
    return nc


# ----------------------------------------------------------------------------
# Launch B: weighted aggregation (alpha precomputed on host), dst-sharded.
# ----------------------------------------------------------------------------

def build_phase_b(sb: int):
    S = sb + SA                # total subtiles per block
    p_b = sb * 128             # B-side (gathered) lanes per block
    npad = BPC * p_b           # B-side lanes per core

    nc = bacc.Bacc("TRN2", target_bir_lowering=False, debug=False,
                   enable_asserts=False, num_devices=NC)
    table = nc.dram_tensor("table", [N, ELEM], GDT, kind="ExternalInput").ap()
    payb = nc.dram_tensor("payb", [BPC * QA, ELEM], GDT, kind="ExternalInput").ap()
    idx_in = nc.dram_tensor("idx16", [128, npad // 16], I16, kind="ExternalInput").ap()
    dcol_c = nc.dram_tensor("dcolc", [128, BPC * S], BF16, kind="ExternalInput").ap()
    alf_in = nc.dram_tensor("alf", [128, BPC * S * H], BF16, kind="ExternalInput").ap()
    iota_r = nc.dram_tensor("iotar", [128, 128], BF16, kind="ExternalInput").ap()
    out = nc.dram_tensor("out", [NPC, H * DH], F32, kind="ExternalOutput").ap()

    with tile.TileContext(nc) as tc, ExitStack() as ctx:
        cpool = ctx.enter_context(tc.tile_pool(name="c", bufs=1))
        gpool = ctx.enter_context(tc.tile_pool(name="g", bufs=3))
        wpool = ctx.enter_context(tc.tile_pool(name="wk", bufs=3))
        opool = ctx.enter_context(tc.tile_pool(name="o", bufs=2))
        pfeat = ctx.enter_context(tc.tile_pool(name="pf", bufs=2, space="PSUM"))

        idx_sb = cpool.tile([128, npad // 16], I16, tag="idx", name="idx")
        nc.sync.dma_start(idx_sb[:], idx_in[:])
        dc_sb = cpool.tile([128, BPC * S], BF16, tag="dc", name="dc")
        nc.sync.dma_start(dc_sb[:], dcol_c[:])
        alf_sb = cpool.tile([128, BPC, S, H], BF16, tag="alf", name="alf")
        nc.sync.dma_start(
            alf_sb[:], alf_in.rearrange("p (b s h) -> p b s h", b=BPC, s=S))
        ior_sb = cpool.tile([128, 128], BF16, tag="ior", name="ior")
        nc.sync.dma_start(ior_sb[:], iota_r[:])

        for b in range(BPC):
            # B-side lanes: indexed gather (Pool desc-gen) into a dedicated
            # contiguous tile — a sliced output AP costs ~40% more ucode
            # time per descriptor.
            gB = gpool.tile([128, sb, ELEM], GDT, tag="gb", name="gb")
            nc.gpsimd.dma_gather(
                gB[:], table[:],
                idx_sb[:, b * (p_b // 16):(b + 1) * (p_b // 16)],
                p_b, p_b, ELEM, single_packet=False)
            # A-side lanes: contiguous payload stream (HWDGE, no desc-gen)
            gA = gpool.tile([128, SA, ELEM], GDT, tag="ga", name="ga")
            nc.sync.dma_start(
                gA[:],
                payb[b * QA:(b + 1) * QA, :].rearrange(
                    "(s p) e -> p s e", p=128))

            # dst one-hot masks: m[e, s, d] = (dcol==d)
            mB = wpool.tile([128, sb, 128], BF16, tag="mb", name="mb")
            nc.vector.tensor_tensor(
                mB[:],
                dc_sb[:, b * S:b * S + sb].unsqueeze(2)
                    .to_broadcast((128, sb, 128)),
                ior_sb[:].unsqueeze(1).to_broadcast((128, sb, 128)),
                op=mybir.AluOpType.is_equal)
            mA = wpool.tile([128, SA, 128], BF16, tag="ma", name="ma")
            nc.vector.tensor_tensor(
                mA[:],
                dc_sb[:, b * S + sb:(b + 1) * S].unsqueeze(2)
                    .to_broadcast((128, SA, 128)),
                ior_sb[:].unsqueeze(1).to_broadcast((128, SA, 128)),
                op=mybir.AluOpType.is_equal)

            # alpha-weighted lanes (padded lanes have alpha=0): broadcast
            # expansion on the otherwise-idle Scalar engine, dense in-place
            # multiply at the DVE's fast tier.
            wtB = wpool.tile([128, sb, H, DH], BF16, tag="wtb", name="wtb")
            nc.scalar.activation(
                wtB[:],
                alf_sb[:, b, 0:sb, :].unsqueeze(3)
                    .to_broadcast((128, sb, H, DH)),
                mybir.ActivationFunctionType.Copy)
            nc.vector.tensor_tensor(
                wtB[:], wtB[:],
                gB[:].rearrange("p s (h d) -> p s h d", d=DH),
                op=mybir.AluOpType.mult)
            wtA = wpool.tile([128, SA, H, DH], BF16, tag="wta", name="wta")
            nc.scalar.activation(
                wtA[:],
                alf_sb[:, b, sb:S, :].unsqueeze(3)
                    .to_broadcast((128, SA, H, DH)),
                mybir.ActivationFunctionType.Copy)
            nc.vector.tensor_tensor(
                wtA[:], wtA[:],
                gA[:].rearrange("p s (h d) -> p s h d", d=DH),
                op=mybir.AluOpType.mult)

            # segment-sum via masked matmul accumulating in PSUM
            pf = pfeat.tile([128, H * DH], F32, tag="pf", name="pf")
            for s in range(sb):
                nc.tensor.matmul(
                    pf[:], mB[:, s, :], wtB[:, s, :, :],
                    start=(s == 0), stop=False)
            for s in range(SA):
                nc.tensor.matmul(
                    pf[:], mA[:, s, :], wtA[:, s, :, :],
                    start=False, stop=(s == SA - 1))

            # epilogue on the idle Scalar engine (bias folded into phase A)
            o_sb = opool.tile([128, H * DH], F32, tag="osb", name="osb")
            nc.scalar.activation(o_sb[:], pf[:],
                                 mybir.ActivationFunctionType.Copy)
            nc.sync.dma_start(out[b * 128:(b + 1) * 128, :], o_sb[:])
    nc.compile()
    return nc


# ----------------------------------------------------------------------------
# Host side
# ----------------------------------------------------------------------------

def _assign_blocks(deg):
    """LPT assignment of nodes to 128-node blocks balancing in-degree sums,
    plus a swap refinement pass to pull the max block sum down to the mean
    (E/NBLK) when possible."""
    import heapq
    order = np.argsort(-deg, kind="stable")
    cap = N // NBLK
    hp = [(0, 0, b) for b in range(NBLK)]
    heapq.heapify(hp)
    asg = np.empty(N, np.int64)
    members = [[] for _ in range(NBLK)]
    for i in order:
        s, c, b = heapq.heappop(hp)
        asg[i] = b
        members[b].append(i)
        c += 1
        s += int(deg[i])
        if c < cap:
            heapq.heappush(hp, (s, c, b))
    bsum = np.zeros(NBLK, np.int64)
    for b in range(NBLK):
        bsum[b] = deg[members[b]].sum()
    target = int(np.ceil(deg.sum() / NBLK))
    for _ in range(512):
        hi = int(bsum.argmax())
        if bsum[hi] <= target:
            break
        lo = int(bsum.argmin())
        need = int(bsum[hi] - target)
        degs_lo = {int(deg[v]): v for v in members[lo]}
        done = False
        for u in members[hi]:
            du = int(deg[u])
            for t in range(min(need, 8), 0, -1):
                v = degs_lo.get(du - t)
                if v is not None and bsum[lo] + t <= target:
                    members[hi].remove(u)
                    members[lo].remove(v)
                    members[hi].append(v)
                    members[lo].append(u)
                    asg[u], asg[v] = lo, hi
                    bsum[hi] -= t
                    bsum[lo] += t
                    done = True
                    break
            if done:
                break
        if not done:
            break
    return asg, bsum


def _preprocess(src, dst):
    deg = np.bincount(dst, minlength=N)
    asg, bsum = _assign_blocks(deg)
    # position within block: arbitrary stable order
    new_id = np.empty(N, np.int64)
    eo_n = np.argsort(asg, kind="stable")
    new_id[eo_n] = np.arange(N)

    s2, d2 = new_id[src], new_id[dst]
    eo = np.argsort(d2, kind="stable")
    s2, d2 = s2[eo], d2[eo]
    E = len(s2)
    eblk = d2 // 128

    sb = int(np.ceil((bsum.max() - QA) / 128))
    p_b = sb * 128
    S = sb + SA

    # A-side selection: per (block, src-core) cell take the first QAC edges
    sc = s2 // NPC
    key = eblk * NC + sc
    ko = np.argsort(key, kind="stable")
    kstarts = np.searchsorted(key[ko], np.arange(NBLK * NC))
    rank = np.empty(E, np.int64)
    rank[ko] = np.arange(E) - kstarts[key[ko]]
    cell_counts = np.bincount(key, minlength=NBLK * NC)
    assert cell_counts.min() >= QAC, cell_counts.min()
    aside = rank < QAC

    # A-side: payload position (per src-core) and lane within block
    pay_pos = eblk * QAC + rank          # valid where aside
    lane = np.empty(E, np.int64)
    lane[aside] = p_b + sc[aside] * QAC + rank[aside]

    # B-side: rank among the block's B-side edges -> lanes [0, p_b)
    bkey = np.where(aside, -1, eblk)
    bo = np.argsort(bkey, kind="stable")
    nA = int(aside.sum())
    bstarts = np.searchsorted(bkey[bo], np.arange(NBLK))
    rankB = np.empty(E, np.int64)
    rankB[bo] = np.arange(E) - bstarts[bkey[bo]]
    assert rankB[~aside].max() < p_b
    lane[~aside] = rankB[~aside]

    flat = eblk * (S * 128) + lane

    bsrc = np.zeros((NBLK, p_b), np.int16)
    bsrc[eblk[~aside], lane[~aside]] = s2[~aside].astype(np.int16)
    bcol = np.full((NBLK, S * 128), 255.0, np.float32)
    bcol[eblk, lane] = (d2 % 128).astype(np.float32)

    return dict(new_id=new_id, s2=s2, d2=d2, sb=sb, S=S, p_b=p_b,
                bsrc=bsrc, bcol=bcol, flat=flat, aside=aside, sc=sc,
                pay_pos=pay_pos)


_CACHE = {}


class _Runner:
    """Cached SPMD runner: jits the bass_exec body once per Bass module."""

    def __init__(self, nc):
        install_neuronx_cc_hook()
        self.nc = nc
        part_name = (nc.partition_id_tensor.name
                     if nc.partition_id_tensor else None)
        in_names, out_names, out_avals, zero_outs = [], [], [], []
        for alloc in nc.m.functions[0].allocations:
            if not isinstance(alloc, mybir.MemoryLocationSet):
                continue
            name = alloc.memorylocations[0].name
            if alloc.kind == "ExternalInput":
                if name != part_name:
                    in_names.append(name)
            elif alloc.kind == "ExternalOutput":
                out_names.append(name)
                shape = tuple(alloc.tensor_shape)
                dtype = mybir.dt.np(alloc.dtype)
                out_avals.append(jax.core.ShapedArray(shape, dtype))
                zero_outs.append(np.zeros(shape, dtype))
        self.in_names, self.out_names = in_names, out_names
        self.out_avals, self.zero_outs = out_avals, zero_outs
        n_params, n_outs = len(in_names), len(out_avals)
        all_names = tuple(in_names + out_names
                          + ([part_name] if part_name else []))
        avals = tuple(out_avals)

        def _body(*args):
            operands = list(args)
            if part_name is not None:
                operands.append(partition_id_tensor())
            outs = _bass_exec_p.bind(
                *operands,
                out_avals=avals,
                in_names=all_names,
                out_names=tuple(out_names),
                lowering_input_output_aliases=(),
                sim_require_finite=True,
                sim_require_nnan=True,
                nc=nc,
            )
            return tuple(outs)

        devices = jax.devices()[:NC]
        self.mesh = Mesh(np.asarray(devices), ("core",))
        in_specs = (PartitionSpec("core"),) * (n_params + n_outs)
        out_specs = (PartitionSpec("core"),) * n_outs
        self.fn = jax.jit(
            shard_map(_body, mesh=self.mesh, in_specs=in_specs,
                      out_specs=out_specs, check_rep=False),
            keep_unused=True)

    def prep(self, in_maps):
        """Concatenate per-core inputs along axis 0 (host)."""
        n_params = len(self.in_names)
        concat_in = [
            np.concatenate([in_maps[c][self.in_names[i]] for c in range(NC)],
                           axis=0)
            for i in range(n_params)]
        concat_zeros = [
            np.zeros((NC * z.shape[0], *z.shape[1:]), z.dtype)
            for z in self.zero_outs]
        return concat_in + concat_zeros

    def run_prepped(self, args):
        return self.fn(*args)

    def run(self, in_maps):
        out_arrs = self.fn(*self.prep(in_maps))
        return [
            {name: np.asarray(out_arrs[i]).reshape(NC, *self.out_avals[i].shape)[c]
             for i, name in enumerate(self.out_names)}
            for c in range(NC)]


def _get_kernels(sb):
    key = (sb, MM_F32R)
    if key not in _CACHE:
        _CACHE[key] = (_Runner(build_phase_a()), _Runner(build_phase_b(sb)))
    return _CACHE[key]


def kernel(text, weight, fc_w, attn_l, attn_r, bias, src, dst):
    text = np.asarray(text, np.float32)
    weight = np.asarray(weight, np.float32)
    fc_w = np.asarray(fc_w, np.float32)
    attn_l = np.asarray(attn_l, np.float32)
    attn_r = np.asarray(attn_r, np.float32)
    bias = np.asarray(bias, np.float32)
    src = np.asarray(src).astype(np.int64)
    dst = np.asarray(dst).astype(np.int64)

    pp = _preprocess(src, dst)
    new_id, s2, d2 = pp["new_id"], pp["s2"], pp["d2"]
    sb, S, p_b = pp["sb"], pp["S"], pp["p_b"]
    bsrc, bcol = pp["bsrc"], pp["bcol"]
    aside, sc, pay_pos = pp["aside"], pp["sc"], pp["pay_pos"]
    orig_for_new = np.empty(N, np.int64)
    orig_for_new[new_id] = np.arange(N)

    run_a, run_b = _get_kernels(sb)

    # --- launch A ---
    attn_cat = np.zeros((DIN, 2 * H), np.float32)
    for h in range(H):
        attn_cat[h * DH:(h + 1) * DH, h] = attn_l[h]
        attn_cat[h * DH:(h + 1) * DH, H + h] = attn_r[h]
    ident = np.eye(128, dtype=np.float32)
    text_flat = text.reshape(N, DIN)
    bias_col = np.ascontiguousarray(bias.reshape(DIN, 1))
    # per-src-core local gather indices in payload order
    gidx_all = np.zeros((NC, NAP), np.int16)
    gidx_all[sc[aside], pay_pos[aside]] = (s2[aside] % NPC).astype(np.int16)
    in_maps_a = []
    for c in range(NC):
        rows = orig_for_new[c * NPC:(c + 1) * NPC]
        textT = np.ascontiguousarray(text_flat[rows].T)
        gp = gidx_all[c].reshape(NAP // 16, 16).T
        gp = np.ascontiguousarray(np.tile(gp, (8, 1)))
        in_maps_a.append({"textT": textT, "weight": weight, "fc_w": fc_w,
                          "attn": attn_cat, "ident": ident, "biasv": bias_col,
                          "gidx": gp})
    res_a = run_a.run(in_maps_a)

    table_full = np.concatenate([r["table"] for r in res_a], axis=0)
    elr_full = np.concatenate([r["elr"] for r in res_a], axis=1)  # [8, N]
    el_full = elr_full[:H].T.astype(np.float32)    # [N, H] (new-id order)
    er_full = elr_full[H:].T.astype(np.float32)
    # payload reassembly: [core, blk, QAC, E] -> per dst-core block-major
    pay_all = np.stack([r["pay"] for r in res_a]).reshape(
        NC, NBLK, QAC, ELEM).transpose(1, 0, 2, 3)   # [NBLK, NC, QAC, ELEM]

    # --- host softmax over edges (sorted by dst) ---
    e_log = el_full[s2] + er_full[d2]                       # [E, H]
    e_log = np.where(e_log > 0, e_log, NEG * e_log)
    seg = np.searchsorted(d2, np.arange(N))                 # segment starts
    emax = np.maximum.reduceat(e_log, seg, axis=0)          # [N, H]
    ex = np.exp(e_log - emax[d2])
    den = np.add.reduceat(ex, seg, axis=0)
    alpha = (ex / den[d2]).astype(np.float32)               # [E, H]
    alf_pad = np.zeros((NBLK * S * 128, H), np.float32)
    alf_pad[pp["flat"]] = alpha
    alf_pad = alf_pad.reshape(NBLK, S, 128, H)

    # --- launch B ---
    iota_row = np.broadcast_to(
        np.arange(128, dtype=np.float32), (128, 128)).astype(ml_dtypes.bfloat16)
    in_maps_b = []
    for c in range(NC):
        blks = range(c * BPC, (c + 1) * BPC)
        idx16 = np.concatenate(
            [bsrc[b].reshape(p_b // 16, 16).T for b in blks], axis=1)
        idx16 = np.ascontiguousarray(np.tile(idx16, (8, 1)))
        dcolc = np.concatenate(
            [bcol[b].reshape(S, 128).T for b in blks], axis=1)
        dcolc = np.ascontiguousarray(dcolc).astype(ml_dtypes.bfloat16)
        alf = np.ascontiguousarray(
            alf_pad[c * BPC:(c + 1) * BPC].transpose(2, 0, 1, 3).reshape(
                128, BPC * S * H)).astype(ml_dtypes.bfloat16)
        payb = np.ascontiguousarray(
            pay_all[c * BPC:(c + 1) * BPC].reshape(BPC * QA, ELEM))
        in_maps_b.append({
            "table": table_full, "payb": payb, "idx16": idx16,
            "dcolc": dcolc, "alf": alf, "iotar": iota_row})
    res_b = run_b.run(in_maps_b)

    out_new = np.concatenate([r["out"] for r in res_b], axis=0)
    result = out_new[new_id].reshape(B, L, H * DH).astype(np.float32)

    global _LAST_ARGS
    _LAST_ARGS = (run_a, in_maps_a, run_b, in_maps_b)
    return result


_LAST_ARGS = None


# revision 23
# speedup vs baseline: 1.8398x; 1.1026x over previous
"""Trainium2 Bass kernel for nn_DglGraphAttentionNetwork (GAT layer over a
random graph, B=16, L=1024, DIN=512, H=4 heads, DH=128).

Strategy (8 NeuronCores, SPMD):
  Launch A (data-parallel over nodes): each core projects its 2048 nodes
    (hidden = text@W, h = hidden@fc_w, el/er attention dots) and emits the
    h rows as a bf16 gather table [2048, 512] plus el/er [8, 2048].
  Host: concatenates the 8 table slices into the full [16384, 512] table
    and computes the per-edge softmax weights alpha from el/er (cheap
    scalar pipeline over the edge list; edges are pre-sorted by dst).
  Launch B (dst-sharded): edges are sorted by destination and grouped into
    128-destination blocks (host balances blocks by relabeling nodes). Each
    core handles 16 blocks: h[src] rows are fetched with indirect dma_gather
    (128 edges across partitions), scaled by the uploaded alpha, and the
    per-destination segment-sum is a masked matmul accumulating in PSUM.
"""

import os
import sys

sys.path.insert(0, "/opt/trn_rl_repo")

from contextlib import ExitStack

import numpy as np
import ml_dtypes

import jax
from jax.sharding import Mesh, PartitionSpec
from jax.experimental.shard_map import shard_map

try:
    jax.config.update("jax_compilation_cache_dir", "/tmp/gat_jax_cache")
    jax.config.update("jax_persistent_cache_min_compile_time_secs", 1.0)
    jax.config.update("jax_persistent_cache_min_entry_size_bytes", -1)
except Exception:
    pass

import concourse.bass as bass
import concourse.bacc as bacc
import concourse.mybir as mybir
import concourse.tile as tile
from concourse import library_config
from concourse import bass2jax
from concourse.bass2jax import _bass_exec_p, install_neuronx_cc_hook, partition_id_tensor

F32 = mybir.dt.float32
F32R = mybir.dt.float32r
BF16 = mybir.dt.bfloat16
I16 = mybir.dt.int16

B, L, DIN = 16, 1024, 512
H, DH = 4, 128
N = B * L           # 16384 nodes
NC = 8              # cores
NPC = N // NC       # 2048 nodes per core
NBLK = 128          # destination blocks of 128 nodes
BPC = NBLK // NC    # 16 blocks per core
NEG = 0.2           # leaky_relu slope
ELEM = 512          # gather-table row: just the h features (bf16 -> 1024B)

# The SWDGE gather descriptor generation on the Pool/Q7 engine runs at only
# ~9 ns/descriptor and is phase B's hard bottleneck. Split it between the two
# launches: phase A (whose Pool engine is idle) pre-gathers, for each dst
# block, SA subtiles' worth of edges whose src falls in the core's own node
# range (from its local table slice), and writes them as a contiguous
# edge-ordered payload. Phase B streams that payload with one cheap
# contiguous DMA per block and only runs desc-gen for the remaining SB
# subtiles.
SA = 6              # payload (phase-A pre-gathered) subtiles per block
QA = SA * 128       # 896 A-side edges per block
QAC = QA // NC      # 112 per (block, src-core) cell
NAP = NBLK * QAC    # 14336 payload rows per phase-A core
ACH = 2048          # phase-A gather chunk (rows per dma_gather)

GDT = BF16
GNP = ml_dtypes.bfloat16
# phase-A matmul dtype: float32r streams at bf16 rate with fp32 operands,
# but requires f32r-rounding producers (see build_phase_a).
MM_F32R = os.environ.get("GAT_MMDT", "f32r") == "f32r"
MMDT = F32R if MM_F32R else F32


# ----------------------------------------------------------------------------
# Launch A: projection. Per core: textT [512, 2048] -> table [2048, 512], elr.
# ----------------------------------------------------------------------------

def build_phase_a():
    nc = bacc.Bacc("TRN2", target_bir_lowering=False, debug=False,
                   enable_asserts=False, num_devices=NC)
    textT = nc.dram_tensor("textT", [DIN, NPC], F32, kind="ExternalInput").ap()
    weight = nc.dram_tensor("weight", [DIN, DIN], F32, kind="ExternalInput").ap()
    fc_w = nc.dram_tensor("fc_w", [DIN, DIN], F32, kind="ExternalInput").ap()
    attn = nc.dram_tensor("attn", [DIN, 2 * H], F32, kind="ExternalInput").ap()
    ident = nc.dram_tensor("ident", [128, 128], F32, kind="ExternalInput").ap()
    bias_in = nc.dram_tensor("biasv", [DIN, 1], F32, kind="ExternalInput").ap()
    gidx_in = nc.dram_tensor("gidx", [128, NAP // 16], I16, kind="ExternalInput").ap()
    table = nc.dram_tensor("table", [NPC, ELEM], GDT, kind="ExternalOutput").ap()
    elr = nc.dram_tensor("elr", [2 * H, NPC], F32, kind="ExternalOutput").ap()
    pay = nc.dram_tensor("pay", [NAP, ELEM], GDT, kind="ExternalOutput").ap()

    KT = DIN // 128  # 4 contraction tiles

    with tile.TileContext(nc) as tc, ExitStack() as ctx:
        wpool = ctx.enter_context(tc.tile_pool(name="w", bufs=1))
        hpool = ctx.enter_context(tc.tile_pool(name="h", bufs=1))
        tpool = ctx.enter_context(tc.tile_pool(name="t", bufs=3))
        gpool = ctx.enter_context(tc.tile_pool(name="ga", bufs=2))
        pmm = ctx.enter_context(tc.tile_pool(name="pmm", bufs=2, space="PSUM"))
        pelr = ctx.enter_context(tc.tile_pool(name="pelr", bufs=2, space="PSUM"))
        ptr = ctx.enter_context(tc.tile_pool(name="ptr", bufs=2, space="PSUM"))

        # Load via DMA then launder through one DVE copy each: fp32 matmuls
        # carry a single sync-wait slot in codegen, so every matmul operand
        # must be produced by the same engine (DVE) rather than by one of
        # the 8 round-robin DMA semaphore lanes.
        w_ld = [wpool.tile([128, DIN], F32, tag=f"wl{i}", name=f"wl{i}") for i in range(KT)]
        fc_ld = [wpool.tile([128, DIN], F32, tag=f"fl{i}", name=f"fl{i}") for i in range(KT)]
        attn_ld = [wpool.tile([128, 2 * H], F32, tag=f"al{i}", name=f"al{i}") for i in range(KT)]
        tT_ld = [wpool.tile([128, NPC], F32, tag=f"tl{i}", name=f"tl{i}") for i in range(KT)]
        id_ld = wpool.tile([128, 128], F32, tag="idl", name="idl")
        for i in range(KT):
            nc.sync.dma_start(w_ld[i][:], weight[i * 128:(i + 1) * 128, :])
            nc.sync.dma_start(fc_ld[i][:], fc_w[i * 128:(i + 1) * 128, :])
            nc.sync.dma_start(attn_ld[i][:], attn[i * 128:(i + 1) * 128, :])
            nc.sync.dma_start(tT_ld[i][:], textT[i * 128:(i + 1) * 128, :])
        nc.sync.dma_start(id_ld[:], ident[:])
        # bias[f] laid out feature-on-partition: col t holds bias[t*128+p]
        bias_sb = wpool.tile([128, KT], F32, tag="bv", name="bv")
        nc.sync.dma_start(bias_sb[:], bias_in.rearrange("(t p) o -> p (t o)", p=128))
        w_sb = [wpool.tile([128, DIN], MMDT, tag=f"w{i}", name=f"w{i}") for i in range(KT)]
        fc_sb = [wpool.tile([128, DIN], MMDT, tag=f"fc{i}", name=f"fc{i}") for i in range(KT)]
        attn_sb = [wpool.tile([128, 2 * H], MMDT, tag=f"at{i}", name=f"at{i}") for i in range(KT)]
        tT_sb = [wpool.tile([128, NPC], MMDT, tag=f"tt{i}", name=f"tt{i}") for i in range(KT)]
        id_sb = wpool.tile([128, 128], MMDT, tag="id", name="id")
        for i in range(KT):
            nc.vector.tensor_copy(w_sb[i][:], w_ld[i][:])
            nc.vector.tensor_copy(fc_sb[i][:], fc_ld[i][:])
            nc.vector.tensor_copy(attn_sb[i][:], attn_ld[i][:])
            nc.vector.tensor_copy(tT_sb[i][:], tT_ld[i][:])
        nc.vector.tensor_copy(id_sb[:], id_ld[:])

        # hiddenT[e, n] = sum_d W[d, e] * textT[d, n]
        h1_sb = [hpool.tile([128, NPC], MMDT, tag=f"h1{i}", name=f"h1{i}") for i in range(KT)]
        for et in range(KT):
            for nch in range(NPC // 512):
                p = pmm.tile([128, 512], F32, tag="pmm", name="pmm")
                for dt in range(KT):
                    nc.tensor.matmul(
                        p[:],
                        w_sb[dt][:, et * 128:(et + 1) * 128],
                        tT_sb[dt][:, nch * 512:(nch + 1) * 512],
                        start=(dt == 0), stop=(dt == KT - 1))
                nc.vector.tensor_copy(h1_sb[et][:, nch * 512:(nch + 1) * 512], p[:])

        # hT[f, n] = sum_e fc_w[e, f] * hiddenT[e, n]
        h2_sb = [hpool.tile([128, NPC], MMDT, tag=f"h2{i}", name=f"h2{i}") for i in range(KT)]
        for ft in range(KT):
            for nch in range(NPC // 512):
                p = pmm.tile([128, 512], F32, tag="pmm", name="pmm")
                for et in range(KT):
                    nc.tensor.matmul(
                        p[:],
                        fc_sb[et][:, ft * 128:(ft + 1) * 128],
                        h1_sb[et][:, nch * 512:(nch + 1) * 512],
                        start=(et == 0), stop=(et == KT - 1))
                # + bias here: out = sum_e alpha_e (h[src]+bias) = agg + bias
                # since softmax weights sum to 1; the el/er shift it also
                # induces is constant per head, which softmax cancels.
                nc.vector.tensor_scalar(
                    h2_sb[ft][:, nch * 512:(nch + 1) * 512], p[:],
                    bias_sb[:, ft:ft + 1], None, op0=mybir.AluOpType.add)

        # elrT[c, n] (c = 4 el heads then 4 er heads)
        elr_sb = hpool.tile([2 * H, NPC], F32, tag="elr", name="elr")
        for nch in range(NPC // 512):
            p = pelr.tile([2 * H, 512], F32, tag="pelr", name="pelr")
            for ft in range(KT):
                nc.tensor.matmul(
                    p[:],
                    attn_sb[ft][:],
                    h2_sb[ft][:, nch * 512:(nch + 1) * 512],
                    start=(ft == 0), stop=(ft == KT - 1))
            nc.vector.tensor_copy(elr_sb[:, nch * 512:(nch + 1) * 512], p[:])
        nc.sync.dma_start(elr[:], elr_sb[:])

        # per 128-node tile: transpose hT into row-major table rows
        for nt in range(NPC // 128):
            tab = tpool.tile([128, ELEM], GDT, tag="tab", name="tab")
            for ft in range(KT):
                pt = ptr.tile([128, 128], MMDT, tag="ptr", name="ptr")
                nc.tensor.transpose(
                    pt[:], h2_sb[ft][:, nt * 128:(nt + 1) * 128], id_sb[:])
                nc.scalar.activation(
                    tab[:, ft * DH:(ft + 1) * DH], pt[:],
                    mybir.ActivationFunctionType.Copy)
            nc.sync.dma_start(table[nt * 128:(nt + 1) * 128, :], tab[:])

        # idx tile for the local payload gather
        gidx_sb = wpool.tile([128, NAP // 16], I16, tag="gix", name="gix")
        nc.sync.dma_start(gidx_sb[:], gidx_in[:])

        # pre-gather the A-side edge payload from the just-written local
        # table slice (this is the half of phase B's desc-gen that rides in
        # phase A's Pool-idle shadow)
        for ch in range(NAP // ACH):
            g_sb = gpool.tile([128, ACH // 128, ELEM], GDT, tag="gch",
                              name="gch")
            nc.gpsimd.dma_gather(
                g_sb[:], table[:],
                gidx_sb[:, ch * (ACH // 16):(ch + 1) * (ACH // 16)],
                ACH, ACH, ELEM, single_packet=False)
            nc.sync.dma_start(
                pay[ch * ACH:(ch + 1) * ACH, :].rearrange(
                    "(s p) e -> p s e", p=128),
                g_sb[:])
    nc.compile()
    return nc


# ----------------------------------------------------------------------------
# Launch B: weighted aggregation (alpha precomputed on host), dst-sharded.
# ----------------------------------------------------------------------------

def build_phase_b(sb: int):
    S = sb + SA                # total subtiles per block
    p_b = sb * 128             # B-side (gathered) lanes per block
    npad = BPC * p_b           # B-side lanes per core

    nc = bacc.Bacc("TRN2", target_bir_lowering=False, debug=False,
                   enable_asserts=False, num_devices=NC)
    table = nc.dram_tensor("table", [N, ELEM], GDT, kind="ExternalInput").ap()
    payb = nc.dram_tensor("payb", [BPC * QA, ELEM], GDT, kind="ExternalInput").ap()
    idx_in = nc.dram_tensor("idx16", [128, npad // 16], I16, kind="ExternalInput").ap()
    dcol_c = nc.dram_tensor("dcolc", [128, BPC * S], BF16, kind="ExternalInput").ap()
    alf_in = nc.dram_tensor("alf", [128, BPC * S * H], BF16, kind="ExternalInput").ap()
    iota_r = nc.dram_tensor("iotar", [128, 128], BF16, kind="ExternalInput").ap()
    out = nc.dram_tensor("out", [NPC, H * DH], F32, kind="ExternalOutput").ap()

    with tile.TileContext(nc) as tc, ExitStack() as ctx:
        cpool = ctx.enter_context(tc.tile_pool(name="c", bufs=1))
        gpool = ctx.enter_context(tc.tile_pool(name="g", bufs=3))
        wpool = ctx.enter_context(tc.tile_pool(name="wk", bufs=3))
        opool = ctx.enter_context(tc.tile_pool(name="o", bufs=2))
        pfeat = ctx.enter_context(tc.tile_pool(name="pf", bufs=2, space="PSUM"))

        idx_sb = cpool.tile([128, npad // 16], I16, tag="idx", name="idx")
        nc.sync.dma_start(idx_sb[:], idx_in[:])
        dc_sb = cpool.tile([128, BPC * S], BF16, tag="dc", name="dc")
        nc.sync.dma_start(dc_sb[:], dcol_c[:])
        alf_sb = cpool.tile([128, BPC, S, H], BF16, tag="alf", name="alf")
        nc.sync.dma_start(
            alf_sb[:], alf_in.rearrange("p (b s h) -> p b s h", b=BPC, s=S))
        ior_sb = cpool.tile([128, 128], BF16, tag="ior", name="ior")
        nc.sync.dma_start(ior_sb[:], iota_r[:])

        for b in range(BPC):
            # B-side lanes: indexed gather (Pool desc-gen) into a dedicated
            # contiguous tile — a sliced output AP costs ~40% more ucode
            # time per descriptor.
            gB = gpool.tile([128, sb, ELEM], GDT, tag="gb", name="gb")
            nc.gpsimd.dma_gather(
                gB[:], table[:],
                idx_sb[:, b * (p_b // 16):(b + 1) * (p_b // 16)],
                p_b, p_b, ELEM, single_packet=False)
            # A-side lanes: contiguous payload stream (HWDGE, no desc-gen)
            gA = gpool.tile([128, SA, ELEM], GDT, tag="ga", name="ga")
            nc.sync.dma_start(
                gA[:],
                payb[b * QA:(b + 1) * QA, :].rearrange(
                    "(s p) e -> p s e", p=128))

            # dst one-hot masks: m[e, s, d] = (dcol==d)
            mB = wpool.tile([128, sb, 128], BF16, tag="mb", name="mb")
            nc.vector.tensor_tensor(
                mB[:],
                dc_sb[:, b * S:b * S + sb].unsqueeze(2)
                    .to_broadcast((128, sb, 128)),
                ior_sb[:].unsqueeze(1).to_broadcast((128, sb, 128)),
                op=mybir.AluOpType.is_equal)
            mA = wpool.tile([128, SA, 128], BF16, tag="ma", name="ma")
            nc.vector.tensor_tensor(
                mA[:],
                dc_sb[:, b * S + sb:(b + 1) * S].unsqueeze(2)
                    .to_broadcast((128, SA, 128)),
                ior_sb[:].unsqueeze(1).to_broadcast((128, SA, 128)),
                op=mybir.AluOpType.is_equal)

            # alpha-weighted lanes (padded lanes have alpha=0): broadcast
            # expansion on the otherwise-idle Scalar engine, dense in-place
            # multiply at the DVE's fast tier.
            wtB = wpool.tile([128, sb, H, DH], BF16, tag="wtb", name="wtb")
            nc.scalar.activation(
                wtB[:],
                alf_sb[:, b, 0:sb, :].unsqueeze(3)
                    .to_broadcast((128, sb, H, DH)),
                mybir.ActivationFunctionType.Copy)
            nc.vector.tensor_tensor(
                wtB[:], wtB[:],
                gB[:].rearrange("p s (h d) -> p s h d", d=DH),
                op=mybir.AluOpType.mult)
            wtA = wpool.tile([128, SA, H, DH], BF16, tag="wta", name="wta")
            nc.scalar.activation(
                wtA[:],
                alf_sb[:, b, sb:S, :].unsqueeze(3)
                    .to_broadcast((128, SA, H, DH)),
                mybir.ActivationFunctionType.Copy)
            nc.vector.tensor_tensor(
                wtA[:], wtA[:],
                gA[:].rearrange("p s (h d) -> p s h d", d=DH),
                op=mybir.AluOpType.mult)

            # segment-sum via masked matmul accumulating in PSUM
            pf = pfeat.tile([128, H * DH], F32, tag="pf", name="pf")
            for s in range(sb):
                nc.tensor.matmul(
                    pf[:], mB[:, s, :], wtB[:, s, :, :],
                    start=(s == 0), stop=False)
            for s in range(SA):
                nc.tensor.matmul(
                    pf[:], mA[:, s, :], wtA[:, s, :, :],
                    start=False, stop=(s == SA - 1))

            # epilogue on the idle Scalar engine (bias folded into phase A)
            o_sb = opool.tile([128, H * DH], F32, tag="osb", name="osb")
            nc.scalar.activation(o_sb[:], pf[:],
                                 mybir.ActivationFunctionType.Copy)
            nc.sync.dma_start(out[b * 128:(b + 1) * 128, :], o_sb[:])
    nc.compile()
    return nc


# ----------------------------------------------------------------------------
# Host side
# ----------------------------------------------------------------------------

def _assign_blocks(deg):
    """LPT assignment of nodes to 128-node blocks balancing in-degree sums,
    plus a swap refinement pass to pull the max block sum down to the mean
    (E/NBLK) when possible."""
    import heapq
    order = np.argsort(-deg, kind="stable")
    cap = N // NBLK
    hp = [(0, 0, b) for b in range(NBLK)]
    heapq.heapify(hp)
    asg = np.empty(N, np.int64)
    members = [[] for _ in range(NBLK)]
    for i in order:
        s, c, b = heapq.heappop(hp)
        asg[i] = b
        members[b].append(i)
        c += 1
        s += int(deg[i])
        if c < cap:
            heapq.heappush(hp, (s, c, b))
    bsum = np.zeros(NBLK, np.int64)
    for b in range(NBLK):
        bsum[b] = deg[members[b]].sum()
    target = int(np.ceil(deg.sum() / NBLK))
    for _ in range(512):
        hi = int(bsum.argmax())
        if bsum[hi] <= target:
            break
        lo = int(bsum.argmin())
        need = int(bsum[hi] - target)
        degs_lo = {int(deg[v]): v for v in members[lo]}
        done = False
        for u in members[hi]:
            du = int(deg[u])
            for t in range(min(need, 8), 0, -1):
                v = degs_lo.get(du - t)
                if v is not None and bsum[lo] + t <= target:
                    members[hi].remove(u)
                    members[lo].remove(v)
                    members[hi].append(v)
                    members[lo].append(u)
                    asg[u], asg[v] = lo, hi
                    bsum[hi] -= t
                    bsum[lo] += t
                    done = True
                    break
            if done:
                break
        if not done:
            break
    return asg, bsum


def _preprocess(src, dst):
    deg = np.bincount(dst, minlength=N)
    asg, bsum = _assign_blocks(deg)
    # position within block: arbitrary stable order
    new_id = np.empty(N, np.int64)
    eo_n = np.argsort(asg, kind="stable")
    new_id[eo_n] = np.arange(N)

    s2, d2 = new_id[src], new_id[dst]
    eo = np.argsort(d2, kind="stable")
    s2, d2 = s2[eo], d2[eo]
    E = len(s2)
    eblk = d2 // 128

    sb = int(np.ceil((bsum.max() - QA) / 128))
    p_b = sb * 128
    S = sb + SA

    # A-side selection: per (block, src-core) cell take the first QAC edges
    sc = s2 // NPC
    key = eblk * NC + sc
    ko = np.argsort(key, kind="stable")
    kstarts = np.searchsorted(key[ko], np.arange(NBLK * NC))
    rank = np.empty(E, np.int64)
    rank[ko] = np.arange(E) - kstarts[key[ko]]
    cell_counts = np.bincount(key, minlength=NBLK * NC)
    assert cell_counts.min() >= QAC, cell_counts.min()
    aside = rank < QAC

    # A-side: payload position (per src-core) and lane within block
    pay_pos = eblk * QAC + rank          # valid where aside
    lane = np.empty(E, np.int64)
    lane[aside] = p_b + sc[aside] * QAC + rank[aside]

    # B-side: rank among the block's B-side edges -> lanes [0, p_b)
    bkey = np.where(aside, -1, eblk)
    bo = np.argsort(bkey, kind="stable")
    nA = int(aside.sum())
    bstarts = np.searchsorted(bkey[bo], np.arange(NBLK))
    rankB = np.empty(E, np.int64)
    rankB[bo] = np.arange(E) - bstarts[bkey[bo]]
    assert rankB[~aside].max() < p_b
    lane[~aside] = rankB[~aside]

    flat = eblk * (S * 128) + lane

    bsrc = np.zeros((NBLK, p_b), np.int16)
    bsrc[eblk[~aside], lane[~aside]] = s2[~aside].astype(np.int16)
    bcol = np.full((NBLK, S * 128), 255.0, np.float32)
    bcol[eblk, lane] = (d2 % 128).astype(np.float32)

    return dict(new_id=new_id, s2=s2, d2=d2, sb=sb, S=S, p_b=p_b,
                bsrc=bsrc, bcol=bcol, flat=flat, aside=aside, sc=sc,
                pay_pos=pay_pos)


_CACHE = {}


class _Runner:
    """Cached SPMD runner: jits the bass_exec body once per Bass module."""

    def __init__(self, nc):
        install_neuronx_cc_hook()
        self.nc = nc
        part_name = (nc.partition_id_tensor.name
                     if nc.partition_id_tensor else None)
        in_names, out_names, out_avals, zero_outs = [], [], [], []
        for alloc in nc.m.functions[0].allocations:
            if not isinstance(alloc, mybir.MemoryLocationSet):
                continue
            name = alloc.memorylocations[0].name
            if alloc.kind == "ExternalInput":
                if name != part_name:
                    in_names.append(name)
            elif alloc.kind == "ExternalOutput":
                out_names.append(name)
                shape = tuple(alloc.tensor_shape)
                dtype = mybir.dt.np(alloc.dtype)
                out_avals.append(jax.core.ShapedArray(shape, dtype))
                zero_outs.append(np.zeros(shape, dtype))
        self.in_names, self.out_names = in_names, out_names
        self.out_avals, self.zero_outs = out_avals, zero_outs
        n_params, n_outs = len(in_names), len(out_avals)
        all_names = tuple(in_names + out_names
                          + ([part_name] if part_name else []))
        avals = tuple(out_avals)

        def _body(*args):
            operands = list(args)
            if part_name is not None:
                operands.append(partition_id_tensor())
            outs = _bass_exec_p.bind(
                *operands,
                out_avals=avals,
                in_names=all_names,
                out_names=tuple(out_names),
                lowering_input_output_aliases=(),
                sim_require_finite=True,
                sim_require_nnan=True,
                nc=nc,
            )
            return tuple(outs)

        devices = jax.devices()[:NC]
        self.mesh = Mesh(np.asarray(devices), ("core",))
        in_specs = (PartitionSpec("core"),) * (n_params + n_outs)
        out_specs = (PartitionSpec("core"),) * n_outs
        self.fn = jax.jit(
            shard_map(_body, mesh=self.mesh, in_specs=in_specs,
                      out_specs=out_specs, check_rep=False),
            keep_unused=True)

    def prep(self, in_maps):
        """Concatenate per-core inputs along axis 0 (host)."""
        n_params = len(self.in_names)
        concat_in = [
            np.concatenate([in_maps[c][self.in_names[i]] for c in range(NC)],
                           axis=0)
            for i in range(n_params)]
        concat_zeros = [
            np.zeros((NC * z.shape[0], *z.shape[1:]), z.dtype)
            for z in self.zero_outs]
        return concat_in + concat_zeros

    def run_prepped(self, args):
        return self.fn(*args)

    def run(self, in_maps):
        out_arrs = self.fn(*self.prep(in_maps))
        return [
            {name: np.asarray(out_arrs[i]).reshape(NC, *self.out_avals[i].shape)[c]
             for i, name in enumerate(self.out_names)}
            for c in range(NC)]


def _get_kernels(sb):
    key = (sb, MM_F32R)
    if key not in _CACHE:
        _CACHE[key] = (_Runner(build_phase_a()), _Runner(build_phase_b(sb)))
    return _CACHE[key]


def kernel(text, weight, fc_w, attn_l, attn_r, bias, src, dst):
    text = np.asarray(text, np.float32)
    weight = np.asarray(weight, np.float32)
    fc_w = np.asarray(fc_w, np.float32)
    attn_l = np.asarray(attn_l, np.float32)
    attn_r = np.asarray(attn_r, np.float32)
    bias = np.asarray(bias, np.float32)
    src = np.asarray(src).astype(np.int64)
    dst = np.asarray(dst).astype(np.int64)

    pp = _preprocess(src, dst)
    new_id, s2, d2 = pp["new_id"], pp["s2"], pp["d2"]
    sb, S, p_b = pp["sb"], pp["S"], pp["p_b"]
    bsrc, bcol = pp["bsrc"], pp["bcol"]
    aside, sc, pay_pos = pp["aside"], pp["sc"], pp["pay_pos"]
    orig_for_new = np.empty(N, np.int64)
    orig_for_new[new_id] = np.arange(N)

    run_a, run_b = _get_kernels(sb)

    # --- launch A ---
    attn_cat = np.zeros((DIN, 2 * H), np.float32)
    for h in range(H):
        attn_cat[h * DH:(h + 1) * DH, h] = attn_l[h]
        attn_cat[h * DH:(h + 1) * DH, H + h] = attn_r[h]
    ident = np.eye(128, dtype=np.float32)
    text_flat = text.reshape(N, DIN)
    bias_col = np.ascontiguousarray(bias.reshape(DIN, 1))
    # per-src-core local gather indices in payload order
    gidx_all = np.zeros((NC, NAP), np.int16)
    gidx_all[sc[aside], pay_pos[aside]] = (s2[aside] % NPC).astype(np.int16)
    in_maps_a = []
    for c in range(NC):
        rows = orig_for_new[c * NPC:(c + 1) * NPC]
        textT = np.ascontiguousarray(text_flat[rows].T)
        gp = gidx_all[c].reshape(NAP // 16, 16).T
        gp = np.ascontiguousarray(np.tile(gp, (8, 1)))
        in_maps_a.append({"textT": textT, "weight": weight, "fc_w": fc_w,
                          "attn": attn_cat, "ident": ident, "biasv": bias_col,
                          "gidx": gp})
    res_a = run_a.run(in_maps_a)

    table_full = np.concatenate([r["table"] for r in res_a], axis=0)
    elr_full = np.concatenate([r["elr"] for r in res_a], axis=1)  # [8, N]
    el_full = elr_full[:H].T.astype(np.float32)    # [N, H] (new-id order)
    er_full = elr_full[H:].T.astype(np.float32)
    # payload reassembly: [core, blk, QAC, E] -> per dst-core block-major
    pay_all = np.stack([r["pay"] for r in res_a]).reshape(
        NC, NBLK, QAC, ELEM).transpose(1, 0, 2, 3)   # [NBLK, NC, QAC, ELEM]

    # --- host softmax over edges (sorted by dst) ---
    e_log = el_full[s2] + er_full[d2]                       # [E, H]
    e_log = np.where(e_log > 0, e_log, NEG * e_log)
    seg = np.searchsorted(d2, np.arange(N))                 # segment starts
    emax = np.maximum.reduceat(e_log, seg, axis=0)          # [N, H]
    ex = np.exp(e_log - emax[d2])
    den = np.add.reduceat(ex, seg, axis=0)
    alpha = (ex / den[d2]).astype(np.float32)               # [E, H]
    alf_pad = np.zeros((NBLK * S * 128, H), np.float32)
    alf_pad[pp["flat"]] = alpha
    alf_pad = alf_pad.reshape(NBLK, S, 128, H)

    # --- launch B ---
    iota_row = np.broadcast_to(
        np.arange(128, dtype=np.float32), (128, 128)).astype(ml_dtypes.bfloat16)
    in_maps_b = []
    for c in range(NC):
        blks = range(c * BPC, (c + 1) * BPC)
        idx16 = np.concatenate(
            [bsrc[b].reshape(p_b // 16, 16).T for b in blks], axis=1)
        idx16 = np.ascontiguousarray(np.tile(idx16, (8, 1)))
        dcolc = np.concatenate(
            [bcol[b].reshape(S, 128).T for b in blks], axis=1)
        dcolc = np.ascontiguousarray(dcolc).astype(ml_dtypes.bfloat16)
        alf = np.ascontiguousarray(
            alf_pad[c * BPC:(c + 1) * BPC].transpose(2, 0, 1, 3).reshape(
                128, BPC * S * H)).astype(ml_dtypes.bfloat16)
        payb = np.ascontiguousarray(
            pay_all[c * BPC:(c + 1) * BPC].reshape(BPC * QA, ELEM))
        in_maps_b.append({
            "table": table_full, "payb": payb, "idx16": idx16,
            "dcolc": dcolc, "alf": alf, "iotar": iota_row})
    res_b = run_b.run(in_maps_b)

    out_new = np.concatenate([r["out"] for r in res_b], axis=0)
    result = out_new[new_id].reshape(B, L, H * DH).astype(np.float32)

    global _LAST_ARGS
    _LAST_ARGS = (run_a, in_maps_a, run_b, in_maps_b)
    return result


_LAST_ARGS = None
